# revision 7
# baseline (speedup 1.0000x reference)
"""Trainium2 Bass kernel v2 for nn_LogOddsPerformanceTransformer.

For each element x:  s = logit(x);  out = bins[clip(floor((s-b0)/step),0,63)]

Post-ACT arithmetic runs in fp16: tensor_scalar gets the DVE 4x perf
mode and the output DMA halves (values round to fp16; norm-rel ~5e-3,
well under the 2e-2 gate).  Magic-number floor in fp16 (1024 has ulp 1):
    t1 = round16(s*inv + C)        -> 1024 + floor(g),  g=(s-b0)/step
    w  = max(t1 - (C+0.5), 1023.5-C)   (f32 scalars; exact on 0.5 grid)
    o  = min(w, 1086.5-C) * step       (single fp16 rounding)

Two modes per column group:
  A: a=Ln(x), b=Ln(1-x) on ACT (fp16 out); s=a-b (TT on DVE 2x or Pool)
  B: r=reciprocal(x) on DVE (f32); s'=Ln(r-1) on ACT via bias AP=-1
     (fp16); the sign of s' folds into -inv.  One ACT pass instead of
     two — B groups go last so the drain isn't gated on a busy ACT.

The plan decouples granularities: fine input DMA segments keep the ACT
ramp fed; ACT instructions are coarse (222-cycle init each); TT/TS/out
run on sub-chunks for smooth downstream cadence and early outs.

Data parallel over 8 cores; per core [128 x 4096] f32 in, fp16 out,
single DRAM tensors, slice DMAs, full-width SBUF stage buffers.
"""

import sys

sys.path.insert(0, "/opt/trn_rl_repo")

from contextlib import ExitStack

import numpy as np

import concourse.bass as bass
import concourse.tile as tile
from concourse import bacc, mybir
from concourse.bass_utils import run_bass_kernel_spmd

N = 4_194_304
NCORES = 8
NPER = N // NCORES  # 524288
P = 128
W = NPER // P  # 4096 columns per core

# --- plan -----------------------------------------------------------------
# groups: mode 'A'|'B'; cols = ACT instruction span; sub = TT/TS/out chunk
# widths within the group; tt: 'v' DVE / 'p' Pool (A only); w_eng 'v'|'p'
# per-sub engine for the w stage.
PLAN = dict(
    in_segs=(256, 512, 384, 384, 512, 512, 512, 512, 512),
    in_eng=("s",) * 9,
    groups=(
        dict(mode="A", sub=(256,), tt="v", w_eng=("p",), o_eng=("p",)),
        dict(mode="A", sub=(512,), tt="v", w_eng=("p",), o_eng=("p",)),
        dict(mode="A", sub=(384, 384), tt="v", w_eng=("p", "p"), o_eng=("p", "p")),
        dict(mode="B", sub=(1024,), recip=(512, 512)),
        dict(mode="B", sub=(1024,), recip=(512, 512)),
        dict(mode="B", sub=(512,), recip=(512,)),
    ),
    out_segs=(256, 512, 384, 384, 1024, 1024, 512),
    out_eng=("s",) * 7,
)
# --------------------------------------------------------------------------

f32 = mybir.dt.float32
f16 = mybir.dt.float16
Alu = mybir.AluOpType
Act = mybir.ActivationFunctionType

_BUILD_CACHE: dict[tuple, object] = {}


def _constants(bins: np.ndarray):
    b64 = bins.astype(np.float64)
    nb = len(bins)
    if nb != 64:
        return None
    step = np.float32((b64[-1] - b64[0]) / (nb - 1))
    inv = np.float32((nb - 1) / (b64[-1] - b64[0]))
    # C = 1024 + (-b0*inv - 0.5): the fp16 round of s*inv + C floors g.
    # For linspace(-6,6,64): -b0*inv = 31.5 so C = 1055.0 exactly.
    C = 1024.0 - float(b64[0]) * float(inv) - 0.5
    if C != float(np.float32(C)) or not (1024.0 < C < 1088.0):
        return None
    uniform = np.allclose(np.diff(b64), (b64[-1] - b64[0]) / (nb - 1), rtol=0, atol=1e-5)
    if not uniform:
        return None
    return (float(step), float(inv), C)


def _engine(nc, code):
    return {"s": nc.sync, "v": nc.vector, "p": nc.gpsimd, "a": nc.scalar}[code]


def _build(step, inv, C, plan=None):
    plan = plan or PLAN
    groups = plan["groups"]
    in_segs = plan["in_segs"]
    out_segs = plan["out_segs"]
    in_eng = plan.get("in_eng", ("s",) * len(in_segs))
    out_eng = plan.get("out_eng", ("s",) * len(out_segs))
    gcols = [sum(g["sub"]) for g in groups]
    assert sum(gcols) == W, (sum(gcols), W)
    assert sum(e[1] if isinstance(e, tuple) else e for e in in_segs) == W
    assert sum(out_segs) == W

    n_kv_q = sum(1 for e in out_eng if e == "k")
    assert n_kv_q <= 4, "ucode MAX_SWDGE_QUEUES=4"
    nc = bacc.Bacc(
        "TRN2",
        target_bir_lowering=False,
        debug=False,
        num_swdge_queues=max(1, n_kv_q),
    )
    x_d = nc.dram_tensor("x", [P, W], f32, kind="ExternalInput").ap()
    # 4-D [batch=1, dhi=1, dho=P, n_ctx=W] so kv_writeback can address it;
    # plain DMA outs use o_d4[0, 0] slices.
    o_d4 = nc.dram_tensor("o", [1, 1, P, W], f16, kind="ExternalOutput").ap()
    n_kv = sum(1 for e in out_eng if e == "k")
    kv_sem = nc.alloc_semaphore("kv_out_sem") if n_kv else None

    with tile.TileContext(nc) as tc, ExitStack() as ctx:
        pool = ctx.enter_context(tc.tile_pool(name="pool", bufs=1))

        need_b = any(g["mode"] == "B" for g in groups)
        cm1 = pool.tile([P, 1], f32, tag="cm1")
        nc.gpsimd.memset(cm1[:], -1.0)
        # Dummy 1-col Ln emitted before any DMA: insert_act_table_loads
        # places the 1283ns natural_log table load here, during the DMA
        # ramp, instead of gating the first real activation on it.
        warm = pool.tile([P, 1], f16, tag="warm")
        nc.scalar.activation(warm[:], cm1[:], Act.Ln, 1.0, -1.0)

        x = pool.tile([P, W], f32, tag="x")
        a = pool.tile([P, W], f16, tag="a")
        b = pool.tile([P, W], f16, tag="b")
        r = pool.tile([P, W], f32, tag="r")
        s = pool.tile([P, W], f16, tag="s")
        t1 = pool.tile([P, W], f16, tag="t1")
        w_ = pool.tile([P, W], f16, tag="w")
        o4 = pool.tile([P, 1, 1, W], f16, tag="o")

        # column-offset index tiles for the kv outs, memset early
        kv_idx = {}
        off = 0
        for k, (wd, eng) in enumerate(zip(out_segs, out_eng)):
            if eng == "k":
                iw = pool.tile([P, 1], mybir.dt.int32, tag=f"oidx{k}")
                nc.gpsimd.memset(iw[:], off)
                kv_idx[k] = iw
            off += wd

        # in_segs entries: width (sequential) or (col_offset, width) for an
        # explicit transfer order — the DMA queue order is free even though
        # column ranges are fixed
        segs = []
        off = 0
        for ent in in_segs:
            if isinstance(ent, tuple):
                segs.append(ent)
            else:
                segs.append((off, ent))
                off += ent
        cov = sorted(segs)
        assert cov[0][0] == 0 and all(
            a + w == b for (a, w), (b, _) in zip(cov, cov[1:])
        ) and cov[-1][0] + cov[-1][1] == W, f"in_segs don't tile [0,{W}): {cov}"
        with tc.high_priority():
            for (start, wd), eng in zip(segs, in_eng):
                sl = (slice(None), slice(start, start + wd))
                _engine(nc, eng).dma_start(x[sl], x_d[sl])

        # kv outs: descriptor-gen (prep) emitted EARLY on a dedicated SWDGE
        # queue each, while pool is idle; the cheap trigger at the out
        # position carries the data dep and skips HWDGE + DGE delay.
        kv_q = {}
        if kv_idx:
            with tc.high_priority():
                off = 0
                for k, (wd, eng) in enumerate(zip(out_segs, out_eng)):
                    sl = (slice(None), slice(off, off + wd))
                    if eng == "k":
                        q = len(kv_q)
                        kv_q[k] = q
                        in4 = o4[(slice(None), slice(None), slice(None)) + sl[1:]]
                        prep = nc.gpsimd.kv_writeback(
                            o_d4, in4, kv_idx[k][:],
                            prepare_only=True, sem=kv_sem, queue_num=q,
                        )
                        # tile's DMASW sem must own on_update[0] (deferred
                        # completion slot in both sims + epilogue wait)
                        prep.ins.sync_info = mybir.SyncInfo(on_wait=[], on_update=[])
                    off += wd

        if plan.get("recips_first"):
            goff = 0
            for g in groups:
                gw = sum(g["sub"])
                if g["mode"] == "B":
                    roff = goff
                    for rw in g["recip"]:
                        rsl = (slice(None), slice(roff, roff + rw))
                        nc.vector.reciprocal(r[rsl], x[rsl])
                        roff += rw
                goff += gw

        goff = 0
        for g in groups:
            gctx = tc.high_priority(offset=g["prio"]) if g.get("prio") else None
            if gctx:
                gctx.__enter__()
            gw = sum(g["sub"])
            gsl = (slice(None), slice(goff, goff + gw))
            if g["mode"] == "A":
                nc.scalar.activation(a[gsl], x[gsl], Act.Ln)
                nc.scalar.activation(b[gsl], x[gsl], Act.Ln, 1.0, -1.0)
            else:
                if not plan.get("recips_first"):
                    roff = goff
                    for rw in g["recip"]:
                        rsl = (slice(None), slice(roff, roff + rw))
                        nc.vector.reciprocal(r[rsl], x[rsl])
                        roff += rw
                # s' = Ln(r - 1) = -s ; sign folds into -inv below
                nc.scalar.activation(s[gsl], r[gsl], Act.Ln, cm1[:, 0:1])
            subs = []
            off = goff
            for i, wd in enumerate(g["sub"]):
                subs.append((i, (slice(None), slice(off, off + wd))))
                off += wd

            def stage_tt():
                for i, sl in subs:
                    if g["mode"] == "A":
                        eng = nc.gpsimd if g.get("tt") == "p" else nc.vector
                        eng.tensor_tensor(s[sl], a[sl], b[sl], Alu.subtract)

            def stage_t1():
                for i, sl in subs:
                    if g["mode"] == "A":
                        nc.vector.tensor_scalar(t1[sl], s[sl], inv, C, Alu.mult, Alu.add)
                    else:
                        nc.vector.tensor_scalar(t1[sl], s[sl], -inv, C, Alu.mult, Alu.add)

            def stage_w():
                # w = max(t1 - (1024+b0i), -b0i) with b0i = C + 0.5 - 1024
                # (f32 scalars, exact 0.5-grid out)
                for i, sl in subs:
                    w_engs = g.get("w_eng")
                    weng = nc.gpsimd if (w_engs and w_engs[i] == "p") else nc.vector
                    weng.tensor_scalar(w_[sl], t1[sl], C + 0.5, 1023.5 - C, Alu.subtract, Alu.max)

            def stage_o():
                # o = min(w, 63-b0i) * step
                for i, sl in subs:
                    o_engs = g.get("o_eng")
                    oeng = nc.gpsimd if (o_engs and o_engs[i] == "p") else nc.vector
                    oeng.tensor_scalar(o4[(slice(None), 0, 0) + sl[1:]], w_[sl], 1086.5 - C, step, Alu.min, Alu.mult)

            if plan.get("stage_major"):
                stage_tt(); stage_t1(); stage_w(); stage_o()
            else:
                for i, sl in subs:
                    subs_one = [(i, sl)]
                    save = subs[:]
                    subs[:] = subs_one
                    stage_tt(); stage_t1(); stage_w(); stage_o()
                    subs[:] = save
            if gctx:
                gctx.__exit__(None, None, None)
            goff += gw

        off = 0
        for k, (wd, eng) in enumerate(zip(out_segs, out_eng)):
            sl = (slice(None), slice(off, off + wd))
            if eng == "k":
                nc.gpsimd.trigger_dma(count=None, queue_num=kv_q[k])
            else:
                _engine(nc, eng).dma_start(o_d4[(0, 0) + sl], o4[(slice(None), 0, 0) + sl[1:]])
            off += wd

    nc.compile()
    return nc


def _freeze(obj):
    if isinstance(obj, dict):
        return tuple(sorted((k, _freeze(v)) for k, v in obj.items()))
    if isinstance(obj, (list, tuple)):
        return tuple(_freeze(v) for v in obj)
    return obj


def build(bins: np.ndarray, plan=None):
    key = _constants(bins)
    if key is None:
        raise NotImplementedError("bins not supported by this kernel")
    full_key = (key, _freeze(plan))
    if full_key not in _BUILD_CACHE:
        _BUILD_CACHE[full_key] = _build(*key, plan=plan)
    return _BUILD_CACHE[full_key]


def make_in_maps(Xs: np.ndarray):
    shards = Xs.reshape(NCORES, P, W)
    return [{"x": shards[c]} for c in range(NCORES)]


def kernel(Xs: np.ndarray, bins: np.ndarray) -> np.ndarray:
    Xs = np.asarray(Xs, dtype=np.float32)
    bins = np.asarray(bins, dtype=np.float32)
    nc = build(bins)
    res = run_bass_kernel_spmd(nc, make_in_maps(Xs), core_ids=list(range(NCORES)))
    out = np.concatenate([r["o"].reshape(-1) for r in res.results])
    return out.astype(np.float32)


# revision 8
# speedup vs baseline: 1.0064x; 1.0064x over previous
"""Trainium2 Bass kernel v2 for nn_LogOddsPerformanceTransformer.

For each element x:  s = logit(x);  out = bins[clip(floor((s-b0)/step),0,63)]

Post-ACT arithmetic runs in fp16: tensor_scalar gets the DVE 4x perf
mode and the output DMA halves (values round to fp16; norm-rel ~5e-3,
well under the 2e-2 gate).  Magic-number floor in fp16 (1024 has ulp 1):
    t1 = round16(s*inv + C)        -> 1024 + floor(g),  g=(s-b0)/step
    w  = max(t1 - (C+0.5), 1023.5-C)   (f32 scalars; exact on 0.5 grid)
    o  = min(w, 1086.5-C) * step       (single fp16 rounding)

Two modes per column group:
  A: a=Ln(x), b=Ln(1-x) on ACT (fp16 out); s=a-b (TT on DVE 2x or Pool)
  B: r=reciprocal(x) on DVE (f32); s'=Ln(r-1) on ACT via bias AP=-1
     (fp16); the sign of s' folds into -inv.  One ACT pass instead of
     two — B groups go last so the drain isn't gated on a busy ACT.

The plan decouples granularities: fine input DMA segments keep the ACT
ramp fed; ACT instructions are coarse (222-cycle init each); TT/TS/out
run on sub-chunks for smooth downstream cadence and early outs.

Data parallel over 8 cores; per core [128 x 4096] f32 in, fp16 out,
single DRAM tensors, slice DMAs, full-width SBUF stage buffers.
"""

import sys

sys.path.insert(0, "/opt/trn_rl_repo")

from contextlib import ExitStack

import numpy as np

import concourse.bass as bass
import concourse.tile as tile
from concourse import bacc, mybir
from concourse.bass_utils import run_bass_kernel_spmd

N = 4_194_304
NCORES = 8
NPER = N // NCORES  # 524288
P = 128
W = NPER // P  # 4096 columns per core

# --- plan -----------------------------------------------------------------
# groups: mode 'A'|'B'; cols = ACT instruction span; sub = TT/TS/out chunk
# widths within the group; tt: 'v' DVE / 'p' Pool (A only); w_eng 'v'|'p'
# per-sub engine for the w stage.
PLAN = dict(
    in_segs=(256, 512, 384, 384, 512, 512, 512, 512, 512),
    in_eng=("s",) * 9,
    groups=(
        dict(mode="A", sub=(256,), tt="v", w_eng=("p",), o_eng=("p",)),
        dict(mode="A", sub=(512,), tt="v", w_eng=("p",), o_eng=("p",)),
        dict(mode="A", sub=(384, 384), tt="v", w_eng=("p", "p"), o_eng=("p", "p")),
        dict(mode="B", sub=(1024,), recip=(512, 512)),
        dict(mode="B", sub=(1024,), recip=(512, 512), ln_sub=(512, 512)),
        dict(mode="B", sub=(512,), recip=(512,)),
    ),
    out_segs=(256, 512, 384, 384, 1024, 1024, 512),
    out_eng=("s",) * 7,
)
# --------------------------------------------------------------------------

f32 = mybir.dt.float32
f16 = mybir.dt.float16
Alu = mybir.AluOpType
Act = mybir.ActivationFunctionType

_BUILD_CACHE: dict[tuple, object] = {}


def _constants(bins: np.ndarray):
    b64 = bins.astype(np.float64)
    nb = len(bins)
    if nb != 64:
        return None
    step = np.float32((b64[-1] - b64[0]) / (nb - 1))
    inv = np.float32((nb - 1) / (b64[-1] - b64[0]))
    # C = 1024 + (-b0*inv - 0.5): the fp16 round of s*inv + C floors g.
    # For linspace(-6,6,64): -b0*inv = 31.5 so C = 1055.0 exactly.
    C = 1024.0 - float(b64[0]) * float(inv) - 0.5
    if C != float(np.float32(C)) or not (1024.0 < C < 1088.0):
        return None
    uniform = np.allclose(np.diff(b64), (b64[-1] - b64[0]) / (nb - 1), rtol=0, atol=1e-5)
    if not uniform:
        return None
    return (float(step), float(inv), C)


def _engine(nc, code):
    return {"s": nc.sync, "v": nc.vector, "p": nc.gpsimd, "a": nc.scalar}[code]


def _build(step, inv, C, plan=None):
    plan = plan or PLAN
    groups = plan["groups"]
    in_segs = plan["in_segs"]
    out_segs = plan["out_segs"]
    in_eng = plan.get("in_eng", ("s",) * len(in_segs))
    out_eng = plan.get("out_eng", ("s",) * len(out_segs))
    gcols = [sum(g["sub"]) for g in groups]
    assert sum(gcols) == W, (sum(gcols), W)
    assert sum(e[1] if isinstance(e, tuple) else e for e in in_segs) == W
    assert sum(out_segs) == W

    n_kv_q = sum(1 for e in out_eng if e == "k")
    assert n_kv_q <= 4, "ucode MAX_SWDGE_QUEUES=4"
    nc = bacc.Bacc(
        "TRN2",
        target_bir_lowering=False,
        debug=False,
        num_swdge_queues=max(1, n_kv_q),
    )
    x_d = nc.dram_tensor("x", [P, W], f32, kind="ExternalInput").ap()
    # 4-D [batch=1, dhi=1, dho=P, n_ctx=W] so kv_writeback can address it;
    # plain DMA outs use o_d4[0, 0] slices.
    o_d4 = nc.dram_tensor("o", [1, 1, P, W], f16, kind="ExternalOutput").ap()
    n_kv = sum(1 for e in out_eng if e == "k")
    kv_sem = nc.alloc_semaphore("kv_out_sem") if n_kv else None

    with tile.TileContext(nc) as tc, ExitStack() as ctx:
        pool = ctx.enter_context(tc.tile_pool(name="pool", bufs=1))

        need_b = any(g["mode"] == "B" for g in groups)
        cm1 = pool.tile([P, 1], f32, tag="cm1")
        nc.gpsimd.memset(cm1[:], -1.0)
        # Dummy 1-col Ln emitted before any DMA: insert_act_table_loads
        # places the 1283ns natural_log table load here, during the DMA
        # ramp, instead of gating the first real activation on it.
        warm = pool.tile([P, 1], f16, tag="warm")
        nc.scalar.activation(warm[:], cm1[:], Act.Ln, 1.0, -1.0)

        x = pool.tile([P, W], f32, tag="x")
        a = pool.tile([P, W], f16, tag="a")
        b = pool.tile([P, W], f16, tag="b")
        r = pool.tile([P, W], f32, tag="r")
        s = pool.tile([P, W], f16, tag="s")
        t1 = pool.tile([P, W], f16, tag="t1")
        w_ = pool.tile([P, W], f16, tag="w")
        o4 = pool.tile([P, 1, 1, W], f16, tag="o")

        # column-offset index tiles for the kv outs, memset early
        kv_idx = {}
        off = 0
        for k, (wd, eng) in enumerate(zip(out_segs, out_eng)):
            if eng == "k":
                iw = pool.tile([P, 1], mybir.dt.int32, tag=f"oidx{k}")
                nc.gpsimd.memset(iw[:], off)
                kv_idx[k] = iw
            off += wd

        # in_segs entries: width (sequential) or (col_offset, width) for an
        # explicit transfer order — the DMA queue order is free even though
        # column ranges are fixed
        segs = []
        off = 0
        for ent in in_segs:
            if isinstance(ent, tuple):
                segs.append(ent)
            else:
                segs.append((off, ent))
                off += ent
        cov = sorted(segs)
        assert cov[0][0] == 0 and all(
            a + w == b for (a, w), (b, _) in zip(cov, cov[1:])
        ) and cov[-1][0] + cov[-1][1] == W, f"in_segs don't tile [0,{W}): {cov}"
        with tc.high_priority():
            for (start, wd), eng in zip(segs, in_eng):
                sl = (slice(None), slice(start, start + wd))
                _engine(nc, eng).dma_start(x[sl], x_d[sl])

        # kv outs: descriptor-gen (prep) emitted EARLY on a dedicated SWDGE
        # queue each, while pool is idle; the cheap trigger at the out
        # position carries the data dep and skips HWDGE + DGE delay.
        kv_q = {}
        if kv_idx:
            with tc.high_priority():
                off = 0
                for k, (wd, eng) in enumerate(zip(out_segs, out_eng)):
                    sl = (slice(None), slice(off, off + wd))
                    if eng == "k":
                        q = len(kv_q)
                        kv_q[k] = q
                        in4 = o4[(slice(None), slice(None), slice(None)) + sl[1:]]
                        prep = nc.gpsimd.kv_writeback(
                            o_d4, in4, kv_idx[k][:],
                            prepare_only=True, sem=kv_sem, queue_num=q,
                        )
                        # tile's DMASW sem must own on_update[0] (deferred
                        # completion slot in both sims + epilogue wait)
                        prep.ins.sync_info = mybir.SyncInfo(on_wait=[], on_update=[])
                    off += wd

        if plan.get("recips_first"):
            goff = 0
            for g in groups:
                gw = sum(g["sub"])
                if g["mode"] == "B":
                    roff = goff
                    for rw in g["recip"]:
                        rsl = (slice(None), slice(roff, roff + rw))
                        nc.vector.reciprocal(r[rsl], x[rsl])
                        roff += rw
                goff += gw

        goff = 0
        for g in groups:
            gctx = tc.high_priority(offset=g["prio"]) if g.get("prio") else None
            if gctx:
                gctx.__enter__()
            gw = sum(g["sub"])
            gsl = (slice(None), slice(goff, goff + gw))
            if g["mode"] == "A":
                nc.scalar.activation(a[gsl], x[gsl], Act.Ln)
                nc.scalar.activation(b[gsl], x[gsl], Act.Ln, 1.0, -1.0)
            else:
                if not plan.get("recips_first"):
                    roff = goff
                    for rw in g["recip"]:
                        rsl = (slice(None), slice(roff, roff + rw))
                        nc.vector.reciprocal(r[rsl], x[rsl])
                        roff += rw
                # s' = Ln(r - 1) = -s ; sign folds into -inv below
                # ln_sub: split the Ln so each piece waits only its recip
                loff = goff
                for lw in g.get("ln_sub", (gw,)):
                    lsl = (slice(None), slice(loff, loff + lw))
                    nc.scalar.activation(s[lsl], r[lsl], Act.Ln, cm1[:, 0:1])
                    loff += lw
            subs = []
            off = goff
            for i, wd in enumerate(g["sub"]):
                subs.append((i, (slice(None), slice(off, off + wd))))
                off += wd

            def stage_tt():
                for i, sl in subs:
                    if g["mode"] == "A":
                        eng = nc.gpsimd if g.get("tt") == "p" else nc.vector
                        eng.tensor_tensor(s[sl], a[sl], b[sl], Alu.subtract)

            def stage_t1():
                for i, sl in subs:
                    if g["mode"] == "A":
                        nc.vector.tensor_scalar(t1[sl], s[sl], inv, C, Alu.mult, Alu.add)
                    else:
                        nc.vector.tensor_scalar(t1[sl], s[sl], -inv, C, Alu.mult, Alu.add)

            def stage_w():
                # w = max(t1 - (1024+b0i), -b0i) with b0i = C + 0.5 - 1024
                # (f32 scalars, exact 0.5-grid out)
                for i, sl in subs:
                    w_engs = g.get("w_eng")
                    weng = nc.gpsimd if (w_engs and w_engs[i] == "p") else nc.vector
                    weng.tensor_scalar(w_[sl], t1[sl], C + 0.5, 1023.5 - C, Alu.subtract, Alu.max)

            def stage_o():
                # o = min(w, 63-b0i) * step
                for i, sl in subs:
                    o_engs = g.get("o_eng")
                    oeng = nc.gpsimd if (o_engs and o_engs[i] == "p") else nc.vector
                    oeng.tensor_scalar(o4[(slice(None), 0, 0) + sl[1:]], w_[sl], 1086.5 - C, step, Alu.min, Alu.mult)

            if plan.get("stage_major"):
                stage_tt(); stage_t1(); stage_w(); stage_o()
            else:
                for i, sl in subs:
                    subs_one = [(i, sl)]
                    save = subs[:]
                    subs[:] = subs_one
                    stage_tt(); stage_t1(); stage_w(); stage_o()
                    subs[:] = save
            if gctx:
                gctx.__exit__(None, None, None)
            goff += gw

        off = 0
        for k, (wd, eng) in enumerate(zip(out_segs, out_eng)):
            sl = (slice(None), slice(off, off + wd))
            if eng == "k":
                nc.gpsimd.trigger_dma(count=None, queue_num=kv_q[k])
            else:
                _engine(nc, eng).dma_start(o_d4[(0, 0) + sl], o4[(slice(None), 0, 0) + sl[1:]])
            off += wd

    nc.compile()
    return nc


def _freeze(obj):
    if isinstance(obj, dict):
        return tuple(sorted((k, _freeze(v)) for k, v in obj.items()))
    if isinstance(obj, (list, tuple)):
        return tuple(_freeze(v) for v in obj)
    return obj


def build(bins: np.ndarray, plan=None):
    key = _constants(bins)
    if key is None:
        raise NotImplementedError("bins not supported by this kernel")
    full_key = (key, _freeze(plan))
    if full_key not in _BUILD_CACHE:
        _BUILD_CACHE[full_key] = _build(*key, plan=plan)
    return _BUILD_CACHE[full_key]


def make_in_maps(Xs: np.ndarray):
    shards = Xs.reshape(NCORES, P, W)
    return [{"x": shards[c]} for c in range(NCORES)]


def kernel(Xs: np.ndarray, bins: np.ndarray) -> np.ndarray:
    Xs = np.asarray(Xs, dtype=np.float32)
    bins = np.asarray(bins, dtype=np.float32)
    nc = build(bins)
    res = run_bass_kernel_spmd(nc, make_in_maps(Xs), core_ids=list(range(NCORES)))
    out = np.concatenate([r["o"].reshape(-1) for r in res.results])
    return out.astype(np.float32)


# revision 9
# speedup vs baseline: 1.0138x; 1.0074x over previous
"""Trainium2 Bass kernel v2 for nn_LogOddsPerformanceTransformer.

For each element x:  s = logit(x);  out = bins[clip(floor((s-b0)/step),0,63)]

Post-ACT arithmetic runs in fp16: tensor_scalar gets the DVE 4x perf
mode and the output DMA halves (values round to fp16; norm-rel ~5e-3,
well under the 2e-2 gate).  Magic-number floor in fp16 (1024 has ulp 1):
    t1 = round16(s*inv + C)        -> 1024 + floor(g),  g=(s-b0)/step
    w  = max(t1 - (C+0.5), 1023.5-C)   (f32 scalars; exact on 0.5 grid)
    o  = min(w, 1086.5-C) * step       (single fp16 rounding)

Two modes per column group:
  A: a=Ln(x), b=Ln(1-x) on ACT (fp16 out); s=a-b (TT on DVE 2x or Pool)
  B: r=reciprocal(x) on DVE (f32); s'=Ln(r-1) on ACT via bias AP=-1
     (fp16); the sign of s' folds into -inv.  One ACT pass instead of
     two — B groups go last so the drain isn't gated on a busy ACT.

The plan decouples granularities: fine input DMA segments keep the ACT
ramp fed; ACT instructions are coarse (222-cycle init each); TT/TS/out
run on sub-chunks for smooth downstream cadence and early outs.

Data parallel over 8 cores; per core [128 x 4096] f32 in, fp16 out,
single DRAM tensors, slice DMAs, full-width SBUF stage buffers.
"""

import sys

sys.path.insert(0, "/opt/trn_rl_repo")

from contextlib import ExitStack

import numpy as np

import concourse.bass as bass
import concourse.tile as tile
from concourse import bacc, mybir
from concourse.bass_utils import run_bass_kernel_spmd

N = 4_194_304
NCORES = 8
NPER = N // NCORES  # 524288
P = 128
W = NPER // P  # 4096 columns per core

# --- plan -----------------------------------------------------------------
# groups: mode 'A'|'B'; cols = ACT instruction span; sub = TT/TS/out chunk
# widths within the group; tt: 'v' DVE / 'p' Pool (A only); w_eng 'v'|'p'
# per-sub engine for the w stage.
PLAN = dict(
    in_segs=(256, 512, 384, 384, 512, 512, 512, 512, 512),
    in_eng=("s",) * 9,
    groups=(
        dict(mode="A", sub=(256,), tt="v", w_eng=("p",), o_eng=("p",)),
        dict(mode="A", sub=(512,), tt="v", w_eng=("p",), o_eng=("p",)),
        dict(mode="A", sub=(384,), tt="v", w_eng=("p",), o_eng=("p",)),
        dict(mode="A", sub=(384,), tt="v", w_eng=("p",), o_eng=("p",)),
        dict(mode="B", sub=(1024,), recip=(512, 512)),
        dict(mode="B", sub=(1024,), recip=(512, 512), ln_sub=(512, 512)),
        dict(mode="B", sub=(512,), recip=(512,)),
    ),
    out_segs=(256, 512, 384, 384, 1024, 1024, 512),
    out_eng=("s",) * 7,
)
# --------------------------------------------------------------------------

f32 = mybir.dt.float32
f16 = mybir.dt.float16
Alu = mybir.AluOpType
Act = mybir.ActivationFunctionType

_BUILD_CACHE: dict[tuple, object] = {}


def _constants(bins: np.ndarray):
    b64 = bins.astype(np.float64)
    nb = len(bins)
    if nb != 64:
        return None
    step = np.float32((b64[-1] - b64[0]) / (nb - 1))
    inv = np.float32((nb - 1) / (b64[-1] - b64[0]))
    # C = 1024 + (-b0*inv - 0.5): the fp16 round of s*inv + C floors g.
    # For linspace(-6,6,64): -b0*inv = 31.5 so C = 1055.0 exactly.
    C = 1024.0 - float(b64[0]) * float(inv) - 0.5
    if C != float(np.float32(C)) or not (1024.0 < C < 1088.0):
        return None
    uniform = np.allclose(np.diff(b64), (b64[-1] - b64[0]) / (nb - 1), rtol=0, atol=1e-5)
    if not uniform:
        return None
    return (float(step), float(inv), C)


def _engine(nc, code):
    return {"s": nc.sync, "v": nc.vector, "p": nc.gpsimd, "a": nc.scalar}[code]


def _build(step, inv, C, plan=None):
    plan = plan or PLAN
    groups = plan["groups"]
    in_segs = plan["in_segs"]
    out_segs = plan["out_segs"]
    in_eng = plan.get("in_eng", ("s",) * len(in_segs))
    out_eng = plan.get("out_eng", ("s",) * len(out_segs))
    gcols = [sum(g["sub"]) for g in groups]
    assert sum(gcols) == W, (sum(gcols), W)
    assert sum(e[1] if isinstance(e, tuple) else e for e in in_segs) == W
    assert sum(out_segs) == W

    n_kv_q = sum(1 for e in out_eng if e == "k")
    assert n_kv_q <= 4, "ucode MAX_SWDGE_QUEUES=4"
    nc = bacc.Bacc(
        "TRN2",
        target_bir_lowering=False,
        debug=False,
        num_swdge_queues=max(1, n_kv_q),
    )
    x_d = nc.dram_tensor("x", [P, W], f32, kind="ExternalInput").ap()
    # 4-D [batch=1, dhi=1, dho=P, n_ctx=W] so kv_writeback can address it;
    # plain DMA outs use o_d4[0, 0] slices.
    o_d4 = nc.dram_tensor("o", [1, 1, P, W], f16, kind="ExternalOutput").ap()
    n_kv = sum(1 for e in out_eng if e == "k")
    kv_sem = nc.alloc_semaphore("kv_out_sem") if n_kv else None

    with tile.TileContext(nc) as tc, ExitStack() as ctx:
        pool = ctx.enter_context(tc.tile_pool(name="pool", bufs=1))

        need_b = any(g["mode"] == "B" for g in groups)
        cm1 = pool.tile([P, 1], f32, tag="cm1")
        nc.gpsimd.memset(cm1[:], -1.0)
        # Dummy 1-col Ln emitted before any DMA: insert_act_table_loads
        # places the 1283ns natural_log table load here, during the DMA
        # ramp, instead of gating the first real activation on it.
        warm = pool.tile([P, 1], f16, tag="warm")
        nc.scalar.activation(warm[:], cm1[:], Act.Ln, 1.0, -1.0)

        x = pool.tile([P, W], f32, tag="x")
        a = pool.tile([P, W], f16, tag="a")
        b = pool.tile([P, W], f16, tag="b")
        r = pool.tile([P, W], f32, tag="r")
        s = pool.tile([P, W], f16, tag="s")
        t1 = pool.tile([P, W], f16, tag="t1")
        w_ = pool.tile([P, W], f16, tag="w")
        o4 = pool.tile([P, 1, 1, W], f16, tag="o")

        # column-offset index tiles for the kv outs, memset early
        kv_idx = {}
        off = 0
        for k, (wd, eng) in enumerate(zip(out_segs, out_eng)):
            if eng == "k":
                iw = pool.tile([P, 1], mybir.dt.int32, tag=f"oidx{k}")
                nc.gpsimd.memset(iw[:], off)
                kv_idx[k] = iw
            off += wd

        # in_segs entries: width (sequential) or (col_offset, width) for an
        # explicit transfer order — the DMA queue order is free even though
        # column ranges are fixed
        segs = []
        off = 0
        for ent in in_segs:
            if isinstance(ent, tuple):
                segs.append(ent)
            else:
                segs.append((off, ent))
                off += ent
        cov = sorted(segs)
        assert cov[0][0] == 0 and all(
            a + w == b for (a, w), (b, _) in zip(cov, cov[1:])
        ) and cov[-1][0] + cov[-1][1] == W, f"in_segs don't tile [0,{W}): {cov}"
        with tc.high_priority():
            for (start, wd), eng in zip(segs, in_eng):
                sl = (slice(None), slice(start, start + wd))
                _engine(nc, eng).dma_start(x[sl], x_d[sl])

        # kv outs: descriptor-gen (prep) emitted EARLY on a dedicated SWDGE
        # queue each, while pool is idle; the cheap trigger at the out
        # position carries the data dep and skips HWDGE + DGE delay.
        kv_q = {}
        if kv_idx:
            with tc.high_priority():
                off = 0
                for k, (wd, eng) in enumerate(zip(out_segs, out_eng)):
                    sl = (slice(None), slice(off, off + wd))
                    if eng == "k":
                        q = len(kv_q)
                        kv_q[k] = q
                        in4 = o4[(slice(None), slice(None), slice(None)) + sl[1:]]
                        prep = nc.gpsimd.kv_writeback(
                            o_d4, in4, kv_idx[k][:],
                            prepare_only=True, sem=kv_sem, queue_num=q,
                        )
                        # tile's DMASW sem must own on_update[0] (deferred
                        # completion slot in both sims + epilogue wait)
                        prep.ins.sync_info = mybir.SyncInfo(on_wait=[], on_update=[])
                    off += wd

        if plan.get("recips_first"):
            goff = 0
            for g in groups:
                gw = sum(g["sub"])
                if g["mode"] == "B":
                    roff = goff
                    for rw in g["recip"]:
                        rsl = (slice(None), slice(roff, roff + rw))
                        nc.vector.reciprocal(r[rsl], x[rsl])
                        roff += rw
                goff += gw

        goff = 0
        for g in groups:
            gctx = tc.high_priority(offset=g["prio"]) if g.get("prio") else None
            if gctx:
                gctx.__enter__()
            gw = sum(g["sub"])
            gsl = (slice(None), slice(goff, goff + gw))
            if g["mode"] == "A":
                nc.scalar.activation(a[gsl], x[gsl], Act.Ln)
                nc.scalar.activation(b[gsl], x[gsl], Act.Ln, 1.0, -1.0)
            else:
                if not plan.get("recips_first"):
                    roff = goff
                    for rw in g["recip"]:
                        rsl = (slice(None), slice(roff, roff + rw))
                        nc.vector.reciprocal(r[rsl], x[rsl])
                        roff += rw
                # s' = Ln(r - 1) = -s ; sign folds into -inv below
                # ln_sub: split the Ln so each piece waits only its recip
                loff = goff
                for lw in g.get("ln_sub", (gw,)):
                    lsl = (slice(None), slice(loff, loff + lw))
                    nc.scalar.activation(s[lsl], r[lsl], Act.Ln, cm1[:, 0:1])
                    loff += lw
            subs = []
            off = goff
            for i, wd in enumerate(g["sub"]):
                subs.append((i, (slice(None), slice(off, off + wd))))
                off += wd

            def stage_tt():
                for i, sl in subs:
                    if g["mode"] == "A":
                        eng = nc.gpsimd if g.get("tt") == "p" else nc.vector
                        eng.tensor_tensor(s[sl], a[sl], b[sl], Alu.subtract)

            def stage_t1():
                for i, sl in subs:
                    if g["mode"] == "A":
                        nc.vector.tensor_scalar(t1[sl], s[sl], inv, C, Alu.mult, Alu.add)
                    else:
                        nc.vector.tensor_scalar(t1[sl], s[sl], -inv, C, Alu.mult, Alu.add)

            def stage_w():
                # w = max(t1 - (1024+b0i), -b0i) with b0i = C + 0.5 - 1024
                # (f32 scalars, exact 0.5-grid out)
                for i, sl in subs:
                    w_engs = g.get("w_eng")
                    weng = nc.gpsimd if (w_engs and w_engs[i] == "p") else nc.vector
                    weng.tensor_scalar(w_[sl], t1[sl], C + 0.5, 1023.5 - C, Alu.subtract, Alu.max)

            def stage_o():
                # o = min(w, 63-b0i) * step
                for i, sl in subs:
                    o_engs = g.get("o_eng")
                    oeng = nc.gpsimd if (o_engs and o_engs[i] == "p") else nc.vector
                    oeng.tensor_scalar(o4[(slice(None), 0, 0) + sl[1:]], w_[sl], 1086.5 - C, step, Alu.min, Alu.mult)

            if plan.get("stage_major"):
                stage_tt(); stage_t1(); stage_w(); stage_o()
            else:
                for i, sl in subs:
                    subs_one = [(i, sl)]
                    save = subs[:]
                    subs[:] = subs_one
                    stage_tt(); stage_t1(); stage_w(); stage_o()
                    subs[:] = save
            if gctx:
                gctx.__exit__(None, None, None)
            goff += gw

        off = 0
        for k, (wd, eng) in enumerate(zip(out_segs, out_eng)):
            sl = (slice(None), slice(off, off + wd))
            if eng == "k":
                nc.gpsimd.trigger_dma(count=None, queue_num=kv_q[k])
            else:
                _engine(nc, eng).dma_start(o_d4[(0, 0) + sl], o4[(slice(None), 0, 0) + sl[1:]])
            off += wd

    nc.compile()
    return nc


def _freeze(obj):
    if isinstance(obj, dict):
        return tuple(sorted((k, _freeze(v)) for k, v in obj.items()))
    if isinstance(obj, (list, tuple)):
        return tuple(_freeze(v) for v in obj)
    return obj


def build(bins: np.ndarray, plan=None):
    key = _constants(bins)
    if key is None:
        raise NotImplementedError("bins not supported by this kernel")
    full_key = (key, _freeze(plan))
    if full_key not in _BUILD_CACHE:
        _BUILD_CACHE[full_key] = _build(*key, plan=plan)
    return _BUILD_CACHE[full_key]


def make_in_maps(Xs: np.ndarray):
    shards = Xs.reshape(NCORES, P, W)
    return [{"x": shards[c]} for c in range(NCORES)]


def kernel(Xs: np.ndarray, bins: np.ndarray) -> np.ndarray:
    Xs = np.asarray(Xs, dtype=np.float32)
    bins = np.asarray(bins, dtype=np.float32)
    nc = build(bins)
    res = run_bass_kernel_spmd(nc, make_in_maps(Xs), core_ids=list(range(NCORES)))
    out = np.concatenate([r["o"].reshape(-1) for r in res.results])
    return out.astype(np.float32)


# revision 10
# speedup vs baseline: 1.0259x; 1.0119x over previous
"""Trainium2 Bass kernel v2 for nn_LogOddsPerformanceTransformer.

For each element x:  s = logit(x);  out = bins[clip(floor((s-b0)/step),0,63)]

Post-ACT arithmetic runs in fp16: tensor_scalar gets the DVE 4x perf
mode and the output DMA halves (values round to fp16; norm-rel ~5e-3,
well under the 2e-2 gate).  Magic-number floor in fp16 (1024 has ulp 1):
    t1 = round16(s*inv + C)        -> 1024 + floor(g),  g=(s-b0)/step
    w  = max(t1 - (C+0.5), 1023.5-C)   (f32 scalars; exact on 0.5 grid)
    o  = min(w, 1086.5-C) * step       (single fp16 rounding)

Two modes per column group:
  A: a=Ln(x), b=Ln(1-x) on ACT (fp16 out); s=a-b (TT on DVE 2x or Pool)
  B: r=reciprocal(x) on DVE (f32); s'=Ln(r-1) on ACT via bias AP=-1
     (fp16); the sign of s' folds into -inv.  One ACT pass instead of
     two — B groups go last so the drain isn't gated on a busy ACT.

The plan decouples granularities: fine input DMA segments keep the ACT
ramp fed; ACT instructions are coarse (222-cycle init each); TT/TS/out
run on sub-chunks for smooth downstream cadence and early outs.

Data parallel over 8 cores; per core [128 x 4096] f32 in, fp16 out,
single DRAM tensors, slice DMAs, full-width SBUF stage buffers.
"""

import sys

sys.path.insert(0, "/opt/trn_rl_repo")

from contextlib import ExitStack

import numpy as np

import concourse.bass as bass
import concourse.tile as tile
from concourse import bacc, mybir
from concourse.bass_utils import run_bass_kernel_spmd

N = 4_194_304
NCORES = 8
NPER = N // NCORES  # 524288
P = 128
W = NPER // P  # 4096 columns per core

# --- plan -----------------------------------------------------------------
# groups: mode 'A'|'B'; cols = ACT instruction span; sub = TT/TS/out chunk
# widths within the group; tt: 'v' DVE / 'p' Pool (A only); w_eng 'v'|'p'
# per-sub engine for the w stage.
PLAN = dict(
    in_segs=(256, 512, 384, 384, 512, 512, 512, 512, 512),
    in_eng=("s",) * 9,
    groups=(
        dict(mode="A", sub=(256,), tt="v", w_eng=("p",), o_eng=("p",)),
        dict(mode="A", sub=(512,), tt="v", w_eng=("p",), o_eng=("p",)),
        dict(mode="A", sub=(384,), tt="v", w_eng=("p",), o_eng=("p",)),
        dict(mode="A", sub=(384,), tt="v", w_eng=("p",), o_eng=("p",)),
        dict(mode="B", sub=(1024,), recip=(512, 512)),
        dict(mode="B", sub=(1024,), recip=(512, 512), ln_sub=(512, 512)),
        dict(mode="B", sub=(512,), recip=(512,)),
    ),
    out_segs=(256, 512, 384, 384, 1024, 1024, 512),
    out_eng=("s",) * 7,
)
# --------------------------------------------------------------------------

f32 = mybir.dt.float32
f16 = mybir.dt.float16
Alu = mybir.AluOpType
Act = mybir.ActivationFunctionType

_BUILD_CACHE: dict[tuple, object] = {}


def _constants(bins: np.ndarray):
    b64 = bins.astype(np.float64)
    nb = len(bins)
    if nb != 64:
        return None
    step = np.float32((b64[-1] - b64[0]) / (nb - 1))
    inv = np.float32((nb - 1) / (b64[-1] - b64[0]))
    # C = 1024 + (-b0*inv - 0.5): the fp16 round of s*inv + C floors g.
    # For linspace(-6,6,64): -b0*inv = 31.5 so C = 1055.0 exactly.
    C = 1024.0 - float(b64[0]) * float(inv) - 0.5
    if C != float(np.float32(C)) or not (1024.0 < C < 1088.0):
        return None
    uniform = np.allclose(np.diff(b64), (b64[-1] - b64[0]) / (nb - 1), rtol=0, atol=1e-5)
    if not uniform:
        return None
    return (float(step), float(inv), C)


def _engine(nc, code):
    return {"s": nc.sync, "v": nc.vector, "p": nc.gpsimd, "a": nc.scalar}[code]


def _build(step, inv, C, plan=None):
    plan = plan or PLAN
    groups = plan["groups"]
    in_segs = plan["in_segs"]
    out_segs = plan["out_segs"]
    in_eng = plan.get("in_eng", ("s",) * len(in_segs))
    out_eng = plan.get("out_eng", ("s",) * len(out_segs))
    gcols = [sum(g["sub"]) for g in groups]
    assert sum(gcols) == W, (sum(gcols), W)
    assert sum(e[1] if isinstance(e, tuple) else e for e in in_segs) == W
    assert sum(out_segs) == W

    n_kv_q = sum(1 for e in out_eng if e == "k")
    assert n_kv_q <= 4, "ucode MAX_SWDGE_QUEUES=4"
    # Bass.__init__ memsets four const APs before the entry barrier; this
    # kernel only reads the f32 0.0/1.0 ones (activation bias), so skip the
    # bf16/u8 memsets — the barrier (and the whole pipeline) starts earlier.
    _orig_memset = bass.BassGpSimd.memset

    def _skip_unused_consts(self, ap, constant):
        nm = getattr(getattr(ap, "tensor", None), "name", "") or ""
        if nm.startswith("const-") and ("bfloat16" in nm or "uint8" in nm):
            return None
        return _orig_memset(self, ap, constant)

    bass.BassGpSimd.memset = _skip_unused_consts
    try:
        nc = bacc.Bacc(
            "TRN2",
            target_bir_lowering=False,
            debug=False,
            num_swdge_queues=max(1, n_kv_q),
        )
    finally:
        bass.BassGpSimd.memset = _orig_memset
    x_d = nc.dram_tensor("x", [P, W], f32, kind="ExternalInput").ap()
    # 4-D [batch=1, dhi=1, dho=P, n_ctx=W] so kv_writeback can address it;
    # plain DMA outs use o_d4[0, 0] slices.
    o_d4 = nc.dram_tensor("o", [1, 1, P, W], f16, kind="ExternalOutput").ap()
    n_kv = sum(1 for e in out_eng if e == "k")
    kv_sem = nc.alloc_semaphore("kv_out_sem") if n_kv else None

    with tile.TileContext(nc) as tc, ExitStack() as ctx:
        pool = ctx.enter_context(tc.tile_pool(name="pool", bufs=1))

        need_b = any(g["mode"] == "B" for g in groups)
        cm1 = pool.tile([P, 1], f32, tag="cm1")
        nc.gpsimd.memset(cm1[:], -1.0)
        # Dummy 1-col Ln emitted before any DMA: insert_act_table_loads
        # places the 1283ns natural_log table load here, during the DMA
        # ramp, instead of gating the first real activation on it.
        warm = pool.tile([P, 1], f16, tag="warm")
        nc.scalar.activation(warm[:], cm1[:], Act.Ln, 1.0, -1.0)

        x = pool.tile([P, W], f32, tag="x")
        a = pool.tile([P, W], f16, tag="a")
        b = pool.tile([P, W], f16, tag="b")
        r = pool.tile([P, W], f32, tag="r")
        s = pool.tile([P, W], f16, tag="s")
        t1 = pool.tile([P, W], f16, tag="t1")
        w_ = pool.tile([P, W], f16, tag="w")
        o4 = pool.tile([P, 1, 1, W], f16, tag="o")

        # column-offset index tiles for the kv outs, memset early
        kv_idx = {}
        off = 0
        for k, (wd, eng) in enumerate(zip(out_segs, out_eng)):
            if eng == "k":
                iw = pool.tile([P, 1], mybir.dt.int32, tag=f"oidx{k}")
                nc.gpsimd.memset(iw[:], off)
                kv_idx[k] = iw
            off += wd

        # in_segs entries: width (sequential) or (col_offset, width) for an
        # explicit transfer order — the DMA queue order is free even though
        # column ranges are fixed
        segs = []
        off = 0
        for ent in in_segs:
            if isinstance(ent, tuple):
                segs.append(ent)
            else:
                segs.append((off, ent))
                off += ent
        cov = sorted(segs)
        assert cov[0][0] == 0 and all(
            a + w == b for (a, w), (b, _) in zip(cov, cov[1:])
        ) and cov[-1][0] + cov[-1][1] == W, f"in_segs don't tile [0,{W}): {cov}"
        with tc.high_priority():
            for (start, wd), eng in zip(segs, in_eng):
                sl = (slice(None), slice(start, start + wd))
                _engine(nc, eng).dma_start(x[sl], x_d[sl])

        # kv outs: descriptor-gen (prep) emitted EARLY on a dedicated SWDGE
        # queue each, while pool is idle; the cheap trigger at the out
        # position carries the data dep and skips HWDGE + DGE delay.
        kv_q = {}
        if kv_idx:
            with tc.high_priority():
                off = 0
                for k, (wd, eng) in enumerate(zip(out_segs, out_eng)):
                    sl = (slice(None), slice(off, off + wd))
                    if eng == "k":
                        q = len(kv_q)
                        kv_q[k] = q
                        in4 = o4[(slice(None), slice(None), slice(None)) + sl[1:]]
                        prep = nc.gpsimd.kv_writeback(
                            o_d4, in4, kv_idx[k][:],
                            prepare_only=True, sem=kv_sem, queue_num=q,
                        )
                        # tile's DMASW sem must own on_update[0] (deferred
                        # completion slot in both sims + epilogue wait)
                        prep.ins.sync_info = mybir.SyncInfo(on_wait=[], on_update=[])
                    off += wd

        if plan.get("recips_first"):
            goff = 0
            for g in groups:
                gw = sum(g["sub"])
                if g["mode"] == "B":
                    roff = goff
                    for rw in g["recip"]:
                        rsl = (slice(None), slice(roff, roff + rw))
                        nc.vector.reciprocal(r[rsl], x[rsl])
                        roff += rw
                goff += gw

        goff = 0
        for g in groups:
            gctx = tc.high_priority(offset=g["prio"]) if g.get("prio") else None
            if gctx:
                gctx.__enter__()
            gw = sum(g["sub"])
            gsl = (slice(None), slice(goff, goff + gw))
            if g["mode"] == "A":
                nc.scalar.activation(a[gsl], x[gsl], Act.Ln)
                nc.scalar.activation(b[gsl], x[gsl], Act.Ln, 1.0, -1.0)
            else:
                if not plan.get("recips_first"):
                    rctx = (
                        tc.high_priority(offset=g["recip_prio"])
                        if g.get("recip_prio")
                        else None
                    )
                    if rctx:
                        rctx.__enter__()
                    roff = goff
                    for rw in g["recip"]:
                        rsl = (slice(None), slice(roff, roff + rw))
                        nc.vector.reciprocal(r[rsl], x[rsl])
                        roff += rw
                    if rctx:
                        rctx.__exit__(None, None, None)
                # s' = Ln(r - 1) = -s ; sign folds into -inv below
                # ln_sub: split the Ln so each piece waits only its recip
                loff = goff
                for lw in g.get("ln_sub", (gw,)):
                    lsl = (slice(None), slice(loff, loff + lw))
                    nc.scalar.activation(s[lsl], r[lsl], Act.Ln, cm1[:, 0:1])
                    loff += lw
            subs = []
            off = goff
            for i, wd in enumerate(g["sub"]):
                subs.append((i, (slice(None), slice(off, off + wd))))
                off += wd

            def stage_tt():
                for i, sl in subs:
                    if g["mode"] == "A":
                        eng = nc.gpsimd if g.get("tt") == "p" else nc.vector
                        eng.tensor_tensor(s[sl], a[sl], b[sl], Alu.subtract)

            def stage_t1():
                for i, sl in subs:
                    if g["mode"] == "A":
                        nc.vector.tensor_scalar(t1[sl], s[sl], inv, C, Alu.mult, Alu.add)
                    else:
                        nc.vector.tensor_scalar(t1[sl], s[sl], -inv, C, Alu.mult, Alu.add)

            def stage_w():
                # w = max(t1 - (1024+b0i), -b0i) with b0i = C + 0.5 - 1024
                # (f32 scalars, exact 0.5-grid out)
                for i, sl in subs:
                    w_engs = g.get("w_eng")
                    weng = nc.gpsimd if (w_engs and w_engs[i] == "p") else nc.vector
                    weng.tensor_scalar(w_[sl], t1[sl], C + 0.5, 1023.5 - C, Alu.subtract, Alu.max)

            def stage_o():
                # o = min(w, 63-b0i) * step
                for i, sl in subs:
                    o_engs = g.get("o_eng")
                    oeng = nc.gpsimd if (o_engs and o_engs[i] == "p") else nc.vector
                    oeng.tensor_scalar(o4[(slice(None), 0, 0) + sl[1:]], w_[sl], 1086.5 - C, step, Alu.min, Alu.mult)

            if plan.get("stage_major"):
                stage_tt(); stage_t1(); stage_w(); stage_o()
            else:
                for i, sl in subs:
                    subs_one = [(i, sl)]
                    save = subs[:]
                    subs[:] = subs_one
                    stage_tt(); stage_t1(); stage_w(); stage_o()
                    subs[:] = save
            if gctx:
                gctx.__exit__(None, None, None)
            goff += gw

        off = 0
        for k, (wd, eng) in enumerate(zip(out_segs, out_eng)):
            sl = (slice(None), slice(off, off + wd))
            if eng == "k":
                nc.gpsimd.trigger_dma(count=None, queue_num=kv_q[k])
            else:
                _engine(nc, eng).dma_start(o_d4[(0, 0) + sl], o4[(slice(None), 0, 0) + sl[1:]])
            off += wd

    nc.compile()
    return nc


def _freeze(obj):
    if isinstance(obj, dict):
        return tuple(sorted((k, _freeze(v)) for k, v in obj.items()))
    if isinstance(obj, (list, tuple)):
        return tuple(_freeze(v) for v in obj)
    return obj


def build(bins: np.ndarray, plan=None):
    key = _constants(bins)
    if key is None:
        raise NotImplementedError("bins not supported by this kernel")
    full_key = (key, _freeze(plan))
    if full_key not in _BUILD_CACHE:
        _BUILD_CACHE[full_key] = _build(*key, plan=plan)
    return _BUILD_CACHE[full_key]


def make_in_maps(Xs: np.ndarray):
    shards = Xs.reshape(NCORES, P, W)
    return [{"x": shards[c]} for c in range(NCORES)]


def kernel(Xs: np.ndarray, bins: np.ndarray) -> np.ndarray:
    Xs = np.asarray(Xs, dtype=np.float32)
    bins = np.asarray(bins, dtype=np.float32)
    nc = build(bins)
    res = run_bass_kernel_spmd(nc, make_in_maps(Xs), core_ids=list(range(NCORES)))
    out = np.concatenate([r["o"].reshape(-1) for r in res.results])
    return out.astype(np.float32)


# revision 11
# speedup vs baseline: 1.0374x; 1.0112x over previous
"""Trainium2 Bass kernel v2 for nn_LogOddsPerformanceTransformer.

For each element x:  s = logit(x);  out = bins[clip(floor((s-b0)/step),0,63)]

Post-ACT arithmetic runs in fp16: tensor_scalar gets the DVE 4x perf
mode and the output DMA halves (values round to fp16; norm-rel ~5e-3,
well under the 2e-2 gate).  Magic-number floor in fp16 (1024 has ulp 1):
    t1 = round16(s*inv + C)        -> 1024 + floor(g),  g=(s-b0)/step
    w  = max(t1 - (C+0.5), 1023.5-C)   (f32 scalars; exact on 0.5 grid)
    o  = min(w, 1086.5-C) * step       (single fp16 rounding)

Two modes per column group:
  A: a=Ln(x), b=Ln(1-x) on ACT (fp16 out); s=a-b (TT on DVE 2x or Pool)
  B: r=reciprocal(x) on DVE (f32); s'=Ln(r-1) on ACT via bias AP=-1
     (fp16); the sign of s' folds into -inv.  One ACT pass instead of
     two — B groups go last so the drain isn't gated on a busy ACT.

The plan decouples granularities: fine input DMA segments keep the ACT
ramp fed; ACT instructions are coarse (222-cycle init each); TT/TS/out
run on sub-chunks for smooth downstream cadence and early outs.

Data parallel over 8 cores; per core [128 x 4096] f32 in, fp16 out,
single DRAM tensors, slice DMAs, full-width SBUF stage buffers.
"""

import sys

sys.path.insert(0, "/opt/trn_rl_repo")

from contextlib import ExitStack

import numpy as np

import concourse.bass as bass
import concourse.tile as tile
from concourse import bacc, mybir
from concourse.bass_utils import run_bass_kernel_spmd

N = 4_194_304
NCORES = 8
NPER = N // NCORES  # 524288
P = 128
W = NPER // P  # 4096 columns per core

# --- plan -----------------------------------------------------------------
# groups: mode 'A'|'B'; cols = ACT instruction span; sub = TT/TS/out chunk
# widths within the group; tt: 'v' DVE / 'p' Pool (A only); w_eng 'v'|'p'
# per-sub engine for the w stage.
PLAN = dict(
    in_segs=(256, 512, 384, 384, 512, 512, 512, 512, 512),
    in_eng=("s",) * 9,
    groups=(
        dict(mode="A", sub=(256,), tt="v", w_eng=("p",), o_eng=("p",)),
        dict(mode="A", sub=(512,), tt="v", w_eng=("p",), o_eng=("p",)),
        dict(mode="A", sub=(384,), tt="v", w_eng=("p",), o_eng=("p",)),
        dict(mode="A", sub=(384,), tt="v", w_eng=("p",), o_eng=("p",)),
        dict(mode="B", sub=(1024,), recip=(512, 512)),
        dict(mode="B", sub=(1024,), recip=(512, 512), ln_sub=(512, 512)),
        dict(mode="B", sub=(512,), recip=(512,)),
    ),
    out_segs=(256, 512, 384, 384, 1024, 1024, 512),
    out_eng=("s",) * 7,
)
# --------------------------------------------------------------------------

f32 = mybir.dt.float32
f16 = mybir.dt.float16
Alu = mybir.AluOpType
Act = mybir.ActivationFunctionType

_BUILD_CACHE: dict[tuple, object] = {}


def _constants(bins: np.ndarray):
    b64 = bins.astype(np.float64)
    nb = len(bins)
    if nb != 64:
        return None
    step = np.float32((b64[-1] - b64[0]) / (nb - 1))
    inv = np.float32((nb - 1) / (b64[-1] - b64[0]))
    # C = 1024 + (-b0*inv - 0.5): the fp16 round of s*inv + C floors g.
    # For linspace(-6,6,64): -b0*inv = 31.5 so C = 1055.0 exactly.
    C = 1024.0 - float(b64[0]) * float(inv) - 0.5
    if C != float(np.float32(C)) or not (1024.0 < C < 1088.0):
        return None
    uniform = np.allclose(np.diff(b64), (b64[-1] - b64[0]) / (nb - 1), rtol=0, atol=1e-5)
    if not uniform:
        return None
    return (float(step), float(inv), C)


def _engine(nc, code):
    return {"s": nc.sync, "v": nc.vector, "p": nc.gpsimd, "a": nc.scalar}[code]


def _build(step, inv, C, plan=None):
    plan = plan or PLAN
    groups = plan["groups"]
    in_segs = plan["in_segs"]
    out_segs = plan["out_segs"]
    in_eng = plan.get("in_eng", ("s",) * len(in_segs))
    out_eng = plan.get("out_eng", ("s",) * len(out_segs))
    gcols = [sum(g["sub"]) for g in groups]
    assert sum(gcols) == W, (sum(gcols), W)
    assert sum(e[1] if isinstance(e, tuple) else e for e in in_segs) == W
    assert sum(out_segs) == W

    n_kv_q = sum(1 for e in out_eng if e == "k")
    assert n_kv_q <= 4, "ucode MAX_SWDGE_QUEUES=4"
    # Bass.__init__ memsets four const APs before the entry barrier; this
    # kernel only reads the f32 0.0/1.0 ones (activation bias), so skip the
    # bf16/u8 memsets — the barrier (and the whole pipeline) starts earlier.
    _orig_memset = bass.BassGpSimd.memset

    def _skip_unused_consts(self, ap, constant):
        nm = getattr(getattr(ap, "tensor", None), "name", "") or ""
        if nm.startswith("const-"):
            return None
        return _orig_memset(self, ap, constant)

    bass.BassGpSimd.memset = _skip_unused_consts
    try:
        nc = bacc.Bacc(
            "TRN2",
            target_bir_lowering=False,
            debug=False,
            num_swdge_queues=max(1, n_kv_q),
        )
    finally:
        bass.BassGpSimd.memset = _orig_memset
    x_d = nc.dram_tensor("x", [P, W], f32, kind="ExternalInput").ap()
    # 4-D [batch=1, dhi=1, dho=P, n_ctx=W] so kv_writeback can address it;
    # plain DMA outs use o_d4[0, 0] slices.
    o_d4 = nc.dram_tensor("o", [1, 1, P, W], f16, kind="ExternalOutput").ap()
    n_kv = sum(1 for e in out_eng if e == "k")
    kv_sem = nc.alloc_semaphore("kv_out_sem") if n_kv else None

    with tile.TileContext(nc) as tc, ExitStack() as ctx:
        pool = ctx.enter_context(tc.tile_pool(name="pool", bufs=1))

        need_b = any(g["mode"] == "B" for g in groups)
        cm1 = pool.tile([P, 1], f32, tag="cm1")
        nc.gpsimd.memset(cm1[:], -1.0)
        # Own bias tiles replace the framework const APs (whose pre-barrier
        # memsets we skipped): these memset in pool's post-barrier idle time.
        c0 = pool.tile([P, 1], f32, tag="c0")
        nc.gpsimd.memset(c0[:], 0.0)
        c1 = pool.tile([P, 1], f32, tag="c1")
        nc.gpsimd.memset(c1[:], 1.0)
        # Dummy 1-col Ln emitted before any DMA: insert_act_table_loads
        # places the 1283ns natural_log table load here, during the DMA
        # ramp, instead of gating the first real activation on it.
        warm = pool.tile([P, 1], f16, tag="warm")
        nc.scalar.activation(warm[:], cm1[:], Act.Ln, c1[:, 0:1], -1.0)

        x = pool.tile([P, W], f32, tag="x")
        a = pool.tile([P, W], f16, tag="a")
        b = pool.tile([P, W], f16, tag="b")
        r = pool.tile([P, W], f32, tag="r")
        s = pool.tile([P, W], f16, tag="s")
        t1 = pool.tile([P, W], f16, tag="t1")
        w_ = pool.tile([P, W], f16, tag="w")
        o4 = pool.tile([P, 1, 1, W], f16, tag="o")

        # column-offset index tiles for the kv outs, memset early
        kv_idx = {}
        off = 0
        for k, (wd, eng) in enumerate(zip(out_segs, out_eng)):
            if eng == "k":
                iw = pool.tile([P, 1], mybir.dt.int32, tag=f"oidx{k}")
                nc.gpsimd.memset(iw[:], off)
                kv_idx[k] = iw
            off += wd

        # in_segs entries: width (sequential) or (col_offset, width) for an
        # explicit transfer order — the DMA queue order is free even though
        # column ranges are fixed
        segs = []
        off = 0
        for ent in in_segs:
            if isinstance(ent, tuple):
                segs.append(ent)
            else:
                segs.append((off, ent))
                off += ent
        cov = sorted(segs)
        assert cov[0][0] == 0 and all(
            a + w == b for (a, w), (b, _) in zip(cov, cov[1:])
        ) and cov[-1][0] + cov[-1][1] == W, f"in_segs don't tile [0,{W}): {cov}"
        with tc.high_priority():
            for (start, wd), eng in zip(segs, in_eng):
                sl = (slice(None), slice(start, start + wd))
                _engine(nc, eng).dma_start(x[sl], x_d[sl])

        # kv outs: descriptor-gen (prep) emitted EARLY on a dedicated SWDGE
        # queue each, while pool is idle; the cheap trigger at the out
        # position carries the data dep and skips HWDGE + DGE delay.
        kv_q = {}
        if kv_idx:
            with tc.high_priority():
                off = 0
                for k, (wd, eng) in enumerate(zip(out_segs, out_eng)):
                    sl = (slice(None), slice(off, off + wd))
                    if eng == "k":
                        q = len(kv_q)
                        kv_q[k] = q
                        in4 = o4[(slice(None), slice(None), slice(None)) + sl[1:]]
                        prep = nc.gpsimd.kv_writeback(
                            o_d4, in4, kv_idx[k][:],
                            prepare_only=True, sem=kv_sem, queue_num=q,
                        )
                        # tile's DMASW sem must own on_update[0] (deferred
                        # completion slot in both sims + epilogue wait)
                        prep.ins.sync_info = mybir.SyncInfo(on_wait=[], on_update=[])
                    off += wd

        if plan.get("recips_first"):
            goff = 0
            for g in groups:
                gw = sum(g["sub"])
                if g["mode"] == "B":
                    roff = goff
                    for rw in g["recip"]:
                        rsl = (slice(None), slice(roff, roff + rw))
                        nc.vector.reciprocal(r[rsl], x[rsl])
                        roff += rw
                goff += gw

        goff = 0
        for g in groups:
            gctx = tc.high_priority(offset=g["prio"]) if g.get("prio") else None
            if gctx:
                gctx.__enter__()
            gw = sum(g["sub"])
            gsl = (slice(None), slice(goff, goff + gw))
            if g["mode"] == "A":
                nc.scalar.activation(a[gsl], x[gsl], Act.Ln, c0[:, 0:1])
                nc.scalar.activation(b[gsl], x[gsl], Act.Ln, c1[:, 0:1], -1.0)
            else:
                if not plan.get("recips_first"):
                    rctx = (
                        tc.high_priority(offset=g["recip_prio"])
                        if g.get("recip_prio")
                        else None
                    )
                    if rctx:
                        rctx.__enter__()
                    roff = goff
                    for rw in g["recip"]:
                        rsl = (slice(None), slice(roff, roff + rw))
                        nc.vector.reciprocal(r[rsl], x[rsl])
                        roff += rw
                    if rctx:
                        rctx.__exit__(None, None, None)
                # s' = Ln(r - 1) = -s ; sign folds into -inv below
                # ln_sub: split the Ln so each piece waits only its recip
                loff = goff
                for lw in g.get("ln_sub", (gw,)):
                    lsl = (slice(None), slice(loff, loff + lw))
                    nc.scalar.activation(s[lsl], r[lsl], Act.Ln, cm1[:, 0:1])
                    loff += lw
            subs = []
            off = goff
            for i, wd in enumerate(g["sub"]):
                subs.append((i, (slice(None), slice(off, off + wd))))
                off += wd

            def stage_tt():
                for i, sl in subs:
                    if g["mode"] == "A":
                        eng = nc.gpsimd if g.get("tt") == "p" else nc.vector
                        eng.tensor_tensor(s[sl], a[sl], b[sl], Alu.subtract)

            def stage_t1():
                for i, sl in subs:
                    if g["mode"] == "A":
                        nc.vector.tensor_scalar(t1[sl], s[sl], inv, C, Alu.mult, Alu.add)
                    else:
                        nc.vector.tensor_scalar(t1[sl], s[sl], -inv, C, Alu.mult, Alu.add)

            def stage_w():
                # w = max(t1 - (1024+b0i), -b0i) with b0i = C + 0.5 - 1024
                # (f32 scalars, exact 0.5-grid out)
                for i, sl in subs:
                    w_engs = g.get("w_eng")
                    weng = nc.gpsimd if (w_engs and w_engs[i] == "p") else nc.vector
                    weng.tensor_scalar(w_[sl], t1[sl], C + 0.5, 1023.5 - C, Alu.subtract, Alu.max)

            def stage_o():
                # o = min(w, 63-b0i) * step
                for i, sl in subs:
                    o_engs = g.get("o_eng")
                    oeng = nc.gpsimd if (o_engs and o_engs[i] == "p") else nc.vector
                    oeng.tensor_scalar(o4[(slice(None), 0, 0) + sl[1:]], w_[sl], 1086.5 - C, step, Alu.min, Alu.mult)

            if plan.get("stage_major"):
                stage_tt(); stage_t1(); stage_w(); stage_o()
            else:
                for i, sl in subs:
                    subs_one = [(i, sl)]
                    save = subs[:]
                    subs[:] = subs_one
                    stage_tt(); stage_t1(); stage_w(); stage_o()
                    subs[:] = save
            if gctx:
                gctx.__exit__(None, None, None)
            goff += gw

        off = 0
        for k, (wd, eng) in enumerate(zip(out_segs, out_eng)):
            sl = (slice(None), slice(off, off + wd))
            if eng == "k":
                nc.gpsimd.trigger_dma(count=None, queue_num=kv_q[k])
            else:
                _engine(nc, eng).dma_start(o_d4[(0, 0) + sl], o4[(slice(None), 0, 0) + sl[1:]])
            off += wd

    nc.compile()
    return nc


def _freeze(obj):
    if isinstance(obj, dict):
        return tuple(sorted((k, _freeze(v)) for k, v in obj.items()))
    if isinstance(obj, (list, tuple)):
        return tuple(_freeze(v) for v in obj)
    return obj


def build(bins: np.ndarray, plan=None):
    key = _constants(bins)
    if key is None:
        raise NotImplementedError("bins not supported by this kernel")
    full_key = (key, _freeze(plan))
    if full_key not in _BUILD_CACHE:
        _BUILD_CACHE[full_key] = _build(*key, plan=plan)
    return _BUILD_CACHE[full_key]


def make_in_maps(Xs: np.ndarray):
    shards = Xs.reshape(NCORES, P, W)
    return [{"x": shards[c]} for c in range(NCORES)]


def kernel(Xs: np.ndarray, bins: np.ndarray) -> np.ndarray:
    Xs = np.asarray(Xs, dtype=np.float32)
    bins = np.asarray(bins, dtype=np.float32)
    nc = build(bins)
    res = run_bass_kernel_spmd(nc, make_in_maps(Xs), core_ids=list(range(NCORES)))
    out = np.concatenate([r["o"].reshape(-1) for r in res.results])
    return out.astype(np.float32)


# revision 12
# speedup vs baseline: 1.0540x; 1.0160x over previous
"""Trainium2 Bass kernel v2 for nn_LogOddsPerformanceTransformer.

For each element x:  s = logit(x);  out = bins[clip(floor((s-b0)/step),0,63)]

Post-ACT arithmetic runs in fp16: tensor_scalar gets the DVE 4x perf
mode and the output DMA halves (values round to fp16; norm-rel ~5e-3,
well under the 2e-2 gate).  Magic-number floor in fp16 (1024 has ulp 1):
    t1 = round16(s*inv + C)        -> 1024 + floor(g),  g=(s-b0)/step
    w  = max(t1 - (C+0.5), 1023.5-C)   (f32 scalars; exact on 0.5 grid)
    o  = min(w, 1086.5-C) * step       (single fp16 rounding)

Two modes per column group:
  A: a=Ln(x), b=Ln(1-x) on ACT (fp16 out); s=a-b (TT on DVE 2x or Pool)
  B: r=reciprocal(x) on DVE (f32); s'=Ln(r-1) on ACT via bias AP=-1
     (fp16); the sign of s' folds into -inv.  One ACT pass instead of
     two — B groups go last so the drain isn't gated on a busy ACT.

The plan decouples granularities: fine input DMA segments keep the ACT
ramp fed; ACT instructions are coarse (222-cycle init each); TT/TS/out
run on sub-chunks for smooth downstream cadence and early outs.

Data parallel over 8 cores; per core [128 x 4096] f32 in, fp16 out,
single DRAM tensors, slice DMAs, full-width SBUF stage buffers.
"""

import sys

sys.path.insert(0, "/opt/trn_rl_repo")

from contextlib import ExitStack

import numpy as np

import concourse.bass as bass
import concourse.tile as tile
from concourse import bacc, mybir
from concourse.bass_utils import run_bass_kernel_spmd

N = 4_194_304
NCORES = 8
NPER = N // NCORES  # 524288
P = 128
W = NPER // P  # 4096 columns per core

# --- plan -----------------------------------------------------------------
# groups: mode 'A'|'B'; cols = ACT instruction span; sub = TT/TS/out chunk
# widths within the group; tt: 'v' DVE / 'p' Pool (A only); w_eng 'v'|'p'
# per-sub engine for the w stage.
PLAN = dict(
    in_segs=(256, 512, 384, 384, 512, 512, 512, 512, 512),
    in_eng=("s",) * 9,
    groups=(
        dict(mode="A", sub=(256,), tt="v", w_eng=("p",), o_eng=("p",)),
        dict(mode="A", sub=(512,), tt="v", w_eng=("p",), o_eng=("p",)),
        dict(mode="A", sub=(384,), tt="v", w_eng=("p",), o_eng=("p",)),
        dict(mode="A", sub=(384,), tt="v", w_eng=("p",), o_eng=("p",)),
        dict(mode="B", sub=(1024,), recip=(512, 512)),
        dict(mode="B", sub=(1024,), recip=(512, 512), ln_sub=(512, 512)),
        dict(mode="B", sub=(512,), recip=(512,)),
    ),
    out_segs=(256, 512, 384, 384, 1024, 1024, 512),
    out_eng=("s",) * 7,
)
# --------------------------------------------------------------------------

f32 = mybir.dt.float32
f16 = mybir.dt.float16
Alu = mybir.AluOpType
Act = mybir.ActivationFunctionType

_BUILD_CACHE: dict[tuple, object] = {}


def _constants(bins: np.ndarray):
    b64 = bins.astype(np.float64)
    nb = len(bins)
    if nb != 64:
        return None
    step = np.float32((b64[-1] - b64[0]) / (nb - 1))
    inv = np.float32((nb - 1) / (b64[-1] - b64[0]))
    # C = 1024 + (-b0*inv - 0.5): the fp16 round of s*inv + C floors g.
    # For linspace(-6,6,64): -b0*inv = 31.5 so C = 1055.0 exactly.
    C = 1024.0 - float(b64[0]) * float(inv) - 0.5
    if C != float(np.float32(C)) or not (1024.0 < C < 1088.0):
        return None
    uniform = np.allclose(np.diff(b64), (b64[-1] - b64[0]) / (nb - 1), rtol=0, atol=1e-5)
    if not uniform:
        return None
    return (float(step), float(inv), C)


def _engine(nc, code):
    return {"s": nc.sync, "v": nc.vector, "p": nc.gpsimd, "a": nc.scalar}[code]


def _build(step, inv, C, plan=None):
    plan = plan or PLAN
    groups = plan["groups"]
    in_segs = plan["in_segs"]
    out_segs = plan["out_segs"]
    in_eng = plan.get("in_eng", ("s",) * len(in_segs))
    out_eng = plan.get("out_eng", ("s",) * len(out_segs))
    gcols = [sum(g["sub"]) for g in groups]
    assert sum(gcols) == W, (sum(gcols), W)
    assert sum(e[1] if isinstance(e, tuple) else e for e in in_segs) == W
    assert sum(out_segs) == W

    n_kv_q = sum(1 for e in out_eng if e == "k")
    assert n_kv_q <= 4, "ucode MAX_SWDGE_QUEUES=4"
    # Bass.__init__ memsets four const APs before the entry barrier; this
    # kernel only reads the f32 0.0/1.0 ones (activation bias), so skip the
    # bf16/u8 memsets — the barrier (and the whole pipeline) starts earlier.
    _orig_memset = bass.BassGpSimd.memset

    def _skip_unused_consts(self, ap, constant):
        nm = getattr(getattr(ap, "tensor", None), "name", "") or ""
        if nm.startswith("const-"):
            return None
        return _orig_memset(self, ap, constant)

    # With no pre-barrier memsets left, the entry all-engine barrier guards
    # nothing — skip it too so the first DMA issues immediately.
    _orig_barrier = bass.Bass.all_engine_barrier

    def _skip_barrier(self, *a, **kw):
        return None

    bass.BassGpSimd.memset = _skip_unused_consts
    bass.Bass.all_engine_barrier = _skip_barrier
    try:
        nc = bacc.Bacc(
            "TRN2",
            target_bir_lowering=False,
            debug=False,
            num_swdge_queues=max(1, n_kv_q),
        )
    finally:
        bass.BassGpSimd.memset = _orig_memset
        bass.Bass.all_engine_barrier = _orig_barrier
    x_d = nc.dram_tensor("x", [P, W], f32, kind="ExternalInput").ap()
    # 4-D [batch=1, dhi=1, dho=P, n_ctx=W] so kv_writeback can address it;
    # plain DMA outs use o_d4[0, 0] slices.
    o_d4 = nc.dram_tensor("o", [1, 1, P, W], f16, kind="ExternalOutput").ap()
    n_kv = sum(1 for e in out_eng if e == "k")
    kv_sem = nc.alloc_semaphore("kv_out_sem") if n_kv else None

    with tile.TileContext(nc) as tc, ExitStack() as ctx:
        pool = ctx.enter_context(tc.tile_pool(name="pool", bufs=1))

        need_b = any(g["mode"] == "B" for g in groups)
        cm1 = pool.tile([P, 1], f32, tag="cm1")
        nc.gpsimd.memset(cm1[:], -1.0)
        # Own bias tiles replace the framework const APs (whose pre-barrier
        # memsets we skipped): these memset in pool's post-barrier idle time.
        c0 = pool.tile([P, 1], f32, tag="c0")
        nc.gpsimd.memset(c0[:], 0.0)
        c1 = pool.tile([P, 1], f32, tag="c1")
        nc.gpsimd.memset(c1[:], 1.0)
        # Dummy 1-col Ln emitted before any DMA: insert_act_table_loads
        # places the 1283ns natural_log table load here, during the DMA
        # ramp, instead of gating the first real activation on it.
        warm = pool.tile([P, 1], f16, tag="warm")
        nc.scalar.activation(warm[:], cm1[:], Act.Ln, c1[:, 0:1], -1.0)

        x = pool.tile([P, W], f32, tag="x")
        a = pool.tile([P, W], f16, tag="a")
        b = pool.tile([P, W], f16, tag="b")
        r = pool.tile([P, W], f32, tag="r")
        s = pool.tile([P, W], f16, tag="s")
        t1 = pool.tile([P, W], f16, tag="t1")
        w_ = pool.tile([P, W], f16, tag="w")
        o4 = pool.tile([P, 1, 1, W], f16, tag="o")

        # column-offset index tiles for the kv outs, memset early
        kv_idx = {}
        off = 0
        for k, (wd, eng) in enumerate(zip(out_segs, out_eng)):
            if eng == "k":
                iw = pool.tile([P, 1], mybir.dt.int32, tag=f"oidx{k}")
                nc.gpsimd.memset(iw[:], off)
                kv_idx[k] = iw
            off += wd

        # in_segs entries: width (sequential) or (col_offset, width) for an
        # explicit transfer order — the DMA queue order is free even though
        # column ranges are fixed
        segs = []
        off = 0
        for ent in in_segs:
            if isinstance(ent, tuple):
                segs.append(ent)
            else:
                segs.append((off, ent))
                off += ent
        cov = sorted(segs)
        assert cov[0][0] == 0 and all(
            a + w == b for (a, w), (b, _) in zip(cov, cov[1:])
        ) and cov[-1][0] + cov[-1][1] == W, f"in_segs don't tile [0,{W}): {cov}"
        with tc.high_priority():
            for (start, wd), eng in zip(segs, in_eng):
                sl = (slice(None), slice(start, start + wd))
                _engine(nc, eng).dma_start(x[sl], x_d[sl])

        # kv outs: descriptor-gen (prep) emitted EARLY on a dedicated SWDGE
        # queue each, while pool is idle; the cheap trigger at the out
        # position carries the data dep and skips HWDGE + DGE delay.
        kv_q = {}
        if kv_idx:
            with tc.high_priority():
                off = 0
                for k, (wd, eng) in enumerate(zip(out_segs, out_eng)):
                    sl = (slice(None), slice(off, off + wd))
                    if eng == "k":
                        q = len(kv_q)
                        kv_q[k] = q
                        in4 = o4[(slice(None), slice(None), slice(None)) + sl[1:]]
                        prep = nc.gpsimd.kv_writeback(
                            o_d4, in4, kv_idx[k][:],
                            prepare_only=True, sem=kv_sem, queue_num=q,
                        )
                        # tile's DMASW sem must own on_update[0] (deferred
                        # completion slot in both sims + epilogue wait)
                        prep.ins.sync_info = mybir.SyncInfo(on_wait=[], on_update=[])
                    off += wd

        if plan.get("recips_first"):
            goff = 0
            for g in groups:
                gw = sum(g["sub"])
                if g["mode"] == "B":
                    roff = goff
                    for rw in g["recip"]:
                        rsl = (slice(None), slice(roff, roff + rw))
                        nc.vector.reciprocal(r[rsl], x[rsl])
                        roff += rw
                goff += gw

        goff = 0
        for g in groups:
            gctx = tc.high_priority(offset=g["prio"]) if g.get("prio") else None
            if gctx:
                gctx.__enter__()
            gw = sum(g["sub"])
            gsl = (slice(None), slice(goff, goff + gw))
            if g["mode"] == "A":
                nc.scalar.activation(a[gsl], x[gsl], Act.Ln, c0[:, 0:1])
                nc.scalar.activation(b[gsl], x[gsl], Act.Ln, c1[:, 0:1], -1.0)
            else:
                if not plan.get("recips_first"):
                    rctx = (
                        tc.high_priority(offset=g["recip_prio"])
                        if g.get("recip_prio")
                        else None
                    )
                    if rctx:
                        rctx.__enter__()
                    roff = goff
                    for rw in g["recip"]:
                        rsl = (slice(None), slice(roff, roff + rw))
                        nc.vector.reciprocal(r[rsl], x[rsl])
                        roff += rw
                    if rctx:
                        rctx.__exit__(None, None, None)
                # s' = Ln(r - 1) = -s ; sign folds into -inv below
                # ln_sub: split the Ln so each piece waits only its recip
                loff = goff
                for lw in g.get("ln_sub", (gw,)):
                    lsl = (slice(None), slice(loff, loff + lw))
                    nc.scalar.activation(s[lsl], r[lsl], Act.Ln, cm1[:, 0:1])
                    loff += lw
            subs = []
            off = goff
            for i, wd in enumerate(g["sub"]):
                subs.append((i, (slice(None), slice(off, off + wd))))
                off += wd

            def stage_tt():
                for i, sl in subs:
                    if g["mode"] == "A":
                        eng = nc.gpsimd if g.get("tt") == "p" else nc.vector
                        eng.tensor_tensor(s[sl], a[sl], b[sl], Alu.subtract)

            def stage_t1():
                for i, sl in subs:
                    if g["mode"] == "A":
                        nc.vector.tensor_scalar(t1[sl], s[sl], inv, C, Alu.mult, Alu.add)
                    else:
                        nc.vector.tensor_scalar(t1[sl], s[sl], -inv, C, Alu.mult, Alu.add)

            def stage_w():
                # w = max(t1 - (1024+b0i), -b0i) with b0i = C + 0.5 - 1024
                # (f32 scalars, exact 0.5-grid out)
                for i, sl in subs:
                    w_engs = g.get("w_eng")
                    weng = nc.gpsimd if (w_engs and w_engs[i] == "p") else nc.vector
                    weng.tensor_scalar(w_[sl], t1[sl], C + 0.5, 1023.5 - C, Alu.subtract, Alu.max)

            def stage_o():
                # o = min(w, 63-b0i) * step
                for i, sl in subs:
                    o_engs = g.get("o_eng")
                    oeng = nc.gpsimd if (o_engs and o_engs[i] == "p") else nc.vector
                    oeng.tensor_scalar(o4[(slice(None), 0, 0) + sl[1:]], w_[sl], 1086.5 - C, step, Alu.min, Alu.mult)

            if plan.get("stage_major"):
                stage_tt(); stage_t1(); stage_w(); stage_o()
            else:
                for i, sl in subs:
                    subs_one = [(i, sl)]
                    save = subs[:]
                    subs[:] = subs_one
                    stage_tt(); stage_t1(); stage_w(); stage_o()
                    subs[:] = save
            if gctx:
                gctx.__exit__(None, None, None)
            goff += gw

        off = 0
        for k, (wd, eng) in enumerate(zip(out_segs, out_eng)):
            sl = (slice(None), slice(off, off + wd))
            if eng == "k":
                nc.gpsimd.trigger_dma(count=None, queue_num=kv_q[k])
            else:
                _engine(nc, eng).dma_start(o_d4[(0, 0) + sl], o4[(slice(None), 0, 0) + sl[1:]])
            off += wd

    nc.compile()
    return nc


def _freeze(obj):
    if isinstance(obj, dict):
        return tuple(sorted((k, _freeze(v)) for k, v in obj.items()))
    if isinstance(obj, (list, tuple)):
        return tuple(_freeze(v) for v in obj)
    return obj


def build(bins: np.ndarray, plan=None):
    key = _constants(bins)
    if key is None:
        raise NotImplementedError("bins not supported by this kernel")
    full_key = (key, _freeze(plan))
    if full_key not in _BUILD_CACHE:
        _BUILD_CACHE[full_key] = _build(*key, plan=plan)
    return _BUILD_CACHE[full_key]


def make_in_maps(Xs: np.ndarray):
    shards = Xs.reshape(NCORES, P, W)
    return [{"x": shards[c]} for c in range(NCORES)]


def kernel(Xs: np.ndarray, bins: np.ndarray) -> np.ndarray:
    Xs = np.asarray(Xs, dtype=np.float32)
    bins = np.asarray(bins, dtype=np.float32)
    nc = build(bins)
    res = run_bass_kernel_spmd(nc, make_in_maps(Xs), core_ids=list(range(NCORES)))
    out = np.concatenate([r["o"].reshape(-1) for r in res.results])
    return out.astype(np.float32)


# revision 13
# speedup vs baseline: 1.0614x; 1.0071x over previous
"""Trainium2 Bass kernel v2 for nn_LogOddsPerformanceTransformer.

For each element x:  s = logit(x);  out = bins[clip(floor((s-b0)/step),0,63)]

Post-ACT arithmetic runs in fp16: tensor_scalar gets the DVE 4x perf
mode and the output DMA halves (values round to fp16; norm-rel ~5e-3,
well under the 2e-2 gate).  Magic-number floor in fp16 (1024 has ulp 1):
    t1 = round16(s*inv + C)        -> 1024 + floor(g),  g=(s-b0)/step
    w  = max(t1 - (C+0.5), 1023.5-C)   (f32 scalars; exact on 0.5 grid)
    o  = min(w, 1086.5-C) * step       (single fp16 rounding)

Two modes per column group:
  A: a=Ln(x), b=Ln(1-x) on ACT (fp16 out); s=a-b (TT on DVE 2x or Pool)
  B: r=reciprocal(x) on DVE (f32); s'=Ln(r-1) on ACT via bias AP=-1
     (fp16); the sign of s' folds into -inv.  One ACT pass instead of
     two — B groups go last so the drain isn't gated on a busy ACT.

The plan decouples granularities: fine input DMA segments keep the ACT
ramp fed; ACT instructions are coarse (222-cycle init each); TT/TS/out
run on sub-chunks for smooth downstream cadence and early outs.

Data parallel over 8 cores; per core [128 x 4096] f32 in, fp16 out,
single DRAM tensors, slice DMAs, full-width SBUF stage buffers.
"""

import sys

sys.path.insert(0, "/opt/trn_rl_repo")

from contextlib import ExitStack

import numpy as np

import concourse.bass as bass
import concourse.tile as tile
from concourse import bacc, mybir
from concourse.bass_utils import run_bass_kernel_spmd

N = 4_194_304
NCORES = 8
NPER = N // NCORES  # 524288
P = 128
W = NPER // P  # 4096 columns per core

# --- plan -----------------------------------------------------------------
# groups: mode 'A'|'B'; cols = ACT instruction span; sub = TT/TS/out chunk
# widths within the group; tt: 'v' DVE / 'p' Pool (A only); w_eng 'v'|'p'
# per-sub engine for the w stage.
PLAN = dict(
    in_segs=(384, 384, 384, 384, 512, 512, 512, 512, 512),
    in_eng=("s",) * 9,
    groups=(
        dict(mode="A", sub=(384,), tt="v", w_eng=("p",), o_eng=("p",)),
        dict(mode="A", sub=(384,), tt="v", w_eng=("p",), o_eng=("p",)),
        dict(mode="A", sub=(384,), tt="v", w_eng=("p",), o_eng=("p",)),
        dict(mode="A", sub=(384,), tt="v", w_eng=("p",), o_eng=("p",)),
        dict(mode="B", sub=(1024,), recip=(512, 512)),
        dict(mode="B", sub=(1024,), recip=(512, 512), ln_sub=(512, 512)),
        dict(mode="B", sub=(512,), recip=(512,)),
    ),
    out_segs=(384, 384, 384, 384, 1024, 1024, 512),
    out_eng=("s",) * 7,
)
# --------------------------------------------------------------------------

f32 = mybir.dt.float32
f16 = mybir.dt.float16
Alu = mybir.AluOpType
Act = mybir.ActivationFunctionType

_BUILD_CACHE: dict[tuple, object] = {}


def _constants(bins: np.ndarray):
    b64 = bins.astype(np.float64)
    nb = len(bins)
    if nb != 64:
        return None
    step = np.float32((b64[-1] - b64[0]) / (nb - 1))
    inv = np.float32((nb - 1) / (b64[-1] - b64[0]))
    # C = 1024 + (-b0*inv - 0.5): the fp16 round of s*inv + C floors g.
    # For linspace(-6,6,64): -b0*inv = 31.5 so C = 1055.0 exactly.
    C = 1024.0 - float(b64[0]) * float(inv) - 0.5
    if C != float(np.float32(C)) or not (1024.0 < C < 1088.0):
        return None
    uniform = np.allclose(np.diff(b64), (b64[-1] - b64[0]) / (nb - 1), rtol=0, atol=1e-5)
    if not uniform:
        return None
    return (float(step), float(inv), C)


def _engine(nc, code):
    return {"s": nc.sync, "v": nc.vector, "p": nc.gpsimd, "a": nc.scalar}[code]


def _build(step, inv, C, plan=None):
    plan = plan or PLAN
    groups = plan["groups"]
    in_segs = plan["in_segs"]
    out_segs = plan["out_segs"]
    in_eng = plan.get("in_eng", ("s",) * len(in_segs))
    out_eng = plan.get("out_eng", ("s",) * len(out_segs))
    gcols = [sum(g["sub"]) for g in groups]
    assert sum(gcols) == W, (sum(gcols), W)
    assert sum(e[1] if isinstance(e, tuple) else e for e in in_segs) == W
    assert sum(out_segs) == W

    n_kv_q = sum(1 for e in out_eng if e == "k")
    assert n_kv_q <= 4, "ucode MAX_SWDGE_QUEUES=4"
    # Bass.__init__ memsets four const APs before the entry barrier; this
    # kernel only reads the f32 0.0/1.0 ones (activation bias), so skip the
    # bf16/u8 memsets — the barrier (and the whole pipeline) starts earlier.
    _orig_memset = bass.BassGpSimd.memset

    def _skip_unused_consts(self, ap, constant):
        nm = getattr(getattr(ap, "tensor", None), "name", "") or ""
        if nm.startswith("const-"):
            return None
        return _orig_memset(self, ap, constant)

    # With no pre-barrier memsets left, the entry all-engine barrier guards
    # nothing — skip it too so the first DMA issues immediately.
    _orig_barrier = bass.Bass.all_engine_barrier

    def _skip_barrier(self, *a, **kw):
        return None

    bass.BassGpSimd.memset = _skip_unused_consts
    bass.Bass.all_engine_barrier = _skip_barrier
    try:
        nc = bacc.Bacc(
            "TRN2",
            target_bir_lowering=False,
            debug=False,
            num_swdge_queues=max(1, n_kv_q),
        )
    finally:
        bass.BassGpSimd.memset = _orig_memset
        bass.Bass.all_engine_barrier = _orig_barrier
    x_d = nc.dram_tensor("x", [P, W], f32, kind="ExternalInput").ap()
    # 4-D [batch=1, dhi=1, dho=P, n_ctx=W] so kv_writeback can address it;
    # plain DMA outs use o_d4[0, 0] slices.
    o_d4 = nc.dram_tensor("o", [1, 1, P, W], f16, kind="ExternalOutput").ap()
    n_kv = sum(1 for e in out_eng if e == "k")
    kv_sem = nc.alloc_semaphore("kv_out_sem") if n_kv else None

    with tile.TileContext(nc) as tc, ExitStack() as ctx:
        pool = ctx.enter_context(tc.tile_pool(name="pool", bufs=1))

        need_b = any(g["mode"] == "B" for g in groups)
        cm1 = pool.tile([P, 1], f32, tag="cm1")
        nc.gpsimd.memset(cm1[:], -1.0)
        # Own bias tiles replace the framework const APs (whose pre-barrier
        # memsets we skipped): these memset in pool's post-barrier idle time.
        c0 = pool.tile([P, 1], f32, tag="c0")
        nc.gpsimd.memset(c0[:], 0.0)
        c1 = pool.tile([P, 1], f32, tag="c1")
        nc.gpsimd.memset(c1[:], 1.0)
        # Dummy 1-col Ln emitted before any DMA: insert_act_table_loads
        # places the 1283ns natural_log table load here, during the DMA
        # ramp, instead of gating the first real activation on it.
        warm = pool.tile([P, 1], f16, tag="warm")
        nc.scalar.activation(warm[:], cm1[:], Act.Ln, c1[:, 0:1], -1.0)

        x = pool.tile([P, W], f32, tag="x")
        a = pool.tile([P, W], f16, tag="a")
        b = pool.tile([P, W], f16, tag="b")
        r = pool.tile([P, W], f32, tag="r")
        s = pool.tile([P, W], f16, tag="s")
        t1 = pool.tile([P, W], f16, tag="t1")
        w_ = pool.tile([P, W], f16, tag="w")
        o4 = pool.tile([P, 1, 1, W], f16, tag="o")

        # column-offset index tiles for the kv outs, memset early
        kv_idx = {}
        off = 0
        for k, (wd, eng) in enumerate(zip(out_segs, out_eng)):
            if eng == "k":
                iw = pool.tile([P, 1], mybir.dt.int32, tag=f"oidx{k}")
                nc.gpsimd.memset(iw[:], off)
                kv_idx[k] = iw
            off += wd

        # in_segs entries: width (sequential) or (col_offset, width) for an
        # explicit transfer order — the DMA queue order is free even though
        # column ranges are fixed
        segs = []
        off = 0
        for ent in in_segs:
            if isinstance(ent, tuple):
                segs.append(ent)
            else:
                segs.append((off, ent))
                off += ent
        cov = sorted(segs)
        assert cov[0][0] == 0 and all(
            a + w == b for (a, w), (b, _) in zip(cov, cov[1:])
        ) and cov[-1][0] + cov[-1][1] == W, f"in_segs don't tile [0,{W}): {cov}"
        with tc.high_priority():
            for (start, wd), eng in zip(segs, in_eng):
                sl = (slice(None), slice(start, start + wd))
                _engine(nc, eng).dma_start(x[sl], x_d[sl])

        # kv outs: descriptor-gen (prep) emitted EARLY on a dedicated SWDGE
        # queue each, while pool is idle; the cheap trigger at the out
        # position carries the data dep and skips HWDGE + DGE delay.
        kv_q = {}
        if kv_idx:
            with tc.high_priority():
                off = 0
                for k, (wd, eng) in enumerate(zip(out_segs, out_eng)):
                    sl = (slice(None), slice(off, off + wd))
                    if eng == "k":
                        q = len(kv_q)
                        kv_q[k] = q
                        in4 = o4[(slice(None), slice(None), slice(None)) + sl[1:]]
                        prep = nc.gpsimd.kv_writeback(
                            o_d4, in4, kv_idx[k][:],
                            prepare_only=True, sem=kv_sem, queue_num=q,
                        )
                        # tile's DMASW sem must own on_update[0] (deferred
                        # completion slot in both sims + epilogue wait)
                        prep.ins.sync_info = mybir.SyncInfo(on_wait=[], on_update=[])
                    off += wd

        if plan.get("recips_first"):
            goff = 0
            for g in groups:
                gw = sum(g["sub"])
                if g["mode"] == "B":
                    roff = goff
                    for rw in g["recip"]:
                        rsl = (slice(None), slice(roff, roff + rw))
                        nc.vector.reciprocal(r[rsl], x[rsl])
                        roff += rw
                goff += gw

        goff = 0
        for g in groups:
            gctx = tc.high_priority(offset=g["prio"]) if g.get("prio") else None
            if gctx:
                gctx.__enter__()
            gw = sum(g["sub"])
            gsl = (slice(None), slice(goff, goff + gw))
            if g["mode"] == "A":
                nc.scalar.activation(a[gsl], x[gsl], Act.Ln, c0[:, 0:1])
                nc.scalar.activation(b[gsl], x[gsl], Act.Ln, c1[:, 0:1], -1.0)
            else:
                if not plan.get("recips_first"):
                    rctx = (
                        tc.high_priority(offset=g["recip_prio"])
                        if g.get("recip_prio")
                        else None
                    )
                    if rctx:
                        rctx.__enter__()
                    roff = goff
                    for rw in g["recip"]:
                        rsl = (slice(None), slice(roff, roff + rw))
                        nc.vector.reciprocal(r[rsl], x[rsl])
                        roff += rw
                    if rctx:
                        rctx.__exit__(None, None, None)
                # s' = Ln(r - 1) = -s ; sign folds into -inv below
                # ln_sub: split the Ln so each piece waits only its recip
                loff = goff
                for lw in g.get("ln_sub", (gw,)):
                    lsl = (slice(None), slice(loff, loff + lw))
                    nc.scalar.activation(s[lsl], r[lsl], Act.Ln, cm1[:, 0:1])
                    loff += lw
            subs = []
            off = goff
            for i, wd in enumerate(g["sub"]):
                subs.append((i, (slice(None), slice(off, off + wd))))
                off += wd

            def stage_tt():
                for i, sl in subs:
                    if g["mode"] == "A":
                        eng = nc.gpsimd if g.get("tt") == "p" else nc.vector
                        eng.tensor_tensor(s[sl], a[sl], b[sl], Alu.subtract)

            def stage_t1():
                for i, sl in subs:
                    if g["mode"] == "A":
                        nc.vector.tensor_scalar(t1[sl], s[sl], inv, C, Alu.mult, Alu.add)
                    else:
                        nc.vector.tensor_scalar(t1[sl], s[sl], -inv, C, Alu.mult, Alu.add)

            def stage_w():
                # w = max(t1 - (1024+b0i), -b0i) with b0i = C + 0.5 - 1024
                # (f32 scalars, exact 0.5-grid out)
                for i, sl in subs:
                    w_engs = g.get("w_eng")
                    weng = nc.gpsimd if (w_engs and w_engs[i] == "p") else nc.vector
                    weng.tensor_scalar(w_[sl], t1[sl], C + 0.5, 1023.5 - C, Alu.subtract, Alu.max)

            def stage_o():
                # o = min(w, 63-b0i) * step
                for i, sl in subs:
                    o_engs = g.get("o_eng")
                    oeng = nc.gpsimd if (o_engs and o_engs[i] == "p") else nc.vector
                    oeng.tensor_scalar(o4[(slice(None), 0, 0) + sl[1:]], w_[sl], 1086.5 - C, step, Alu.min, Alu.mult)

            if plan.get("stage_major"):
                stage_tt(); stage_t1(); stage_w(); stage_o()
            else:
                for i, sl in subs:
                    subs_one = [(i, sl)]
                    save = subs[:]
                    subs[:] = subs_one
                    stage_tt(); stage_t1(); stage_w(); stage_o()
                    subs[:] = save
            if gctx:
                gctx.__exit__(None, None, None)
            goff += gw

        off = 0
        for k, (wd, eng) in enumerate(zip(out_segs, out_eng)):
            sl = (slice(None), slice(off, off + wd))
            if eng == "k":
                nc.gpsimd.trigger_dma(count=None, queue_num=kv_q[k])
            else:
                _engine(nc, eng).dma_start(o_d4[(0, 0) + sl], o4[(slice(None), 0, 0) + sl[1:]])
            off += wd

    nc.compile()
    return nc


def _freeze(obj):
    if isinstance(obj, dict):
        return tuple(sorted((k, _freeze(v)) for k, v in obj.items()))
    if isinstance(obj, (list, tuple)):
        return tuple(_freeze(v) for v in obj)
    return obj


def build(bins: np.ndarray, plan=None):
    key = _constants(bins)
    if key is None:
        raise NotImplementedError("bins not supported by this kernel")
    full_key = (key, _freeze(plan))
    if full_key not in _BUILD_CACHE:
        _BUILD_CACHE[full_key] = _build(*key, plan=plan)
    return _BUILD_CACHE[full_key]


def make_in_maps(Xs: np.ndarray):
    shards = Xs.reshape(NCORES, P, W)
    return [{"x": shards[c]} for c in range(NCORES)]


def kernel(Xs: np.ndarray, bins: np.ndarray) -> np.ndarray:
    Xs = np.asarray(Xs, dtype=np.float32)
    bins = np.asarray(bins, dtype=np.float32)
    nc = build(bins)
    res = run_bass_kernel_spmd(nc, make_in_maps(Xs), core_ids=list(range(NCORES)))
    out = np.concatenate([r["o"].reshape(-1) for r in res.results])
    return out.astype(np.float32)
